# revision 2
# baseline (speedup 1.0000x reference)
"""Distributed GCN (3x GCNConv + global mean pool + linear) on 8 TRN2 cores.

Sharding: nodes partitioned contiguously across 8 cores; edges partitioned by
dst owner; per-layer node features (bf16) all-gathered to a replicated table
in each core's DRAM; per-edge features fetched with dma_gather (SWDGE);
segment-sum over dst done with precomputed *weighted* one-hot tiles (edge
norm dinv_src*dinv_dst baked in, self-loops appended as edges) streamed from
DRAM via HWDGE.  Layers 1-2 use the reversed matmul form (out [feat, node])
so the aggregation output directly feeds the next layer's lhsT; layer 3 uses
the forward form so pooling can consume [node, feat] tiles via a single
PSUM-accumulated one-hot matmul chain.  Epilogues run on the Scalar engine
(PSUM -> SBUF relu + cast).  The Vector engine is idle in steady state, so
SWDGE descriptor generation never contends for the shared SBUF port pair.
"""
import math
import numpy as np
import ml_dtypes
from contextlib import ExitStack

import concourse.bacc as bacc
import concourse.mybir as mybir
from concourse.tile import TileContext
from concourse.bass_utils import run_bass_kernel_spmd

P = 128
NCORES = 8
N = 100000
E = 1600000
H = 128
C = 10
G = 128
NP = N // NCORES            # 12500 nodes per core
NW = math.ceil(NP / P)      # 98 dst windows per core
NPAD = NW * P               # 12544 padded nodes per core
CHROWS = 25000              # gather chunk rows (int16 idx limit 32767)
NCH = math.ceil(N / CHROWS)  # 4
NI = 1024                   # indices per dma_gather call
OHK = 16                    # one-hot groups fetched per DMA
NLAYERS = 3                 # debug knob

BF16 = ml_dtypes.bfloat16

TRACE = False               # set by test.py for profiling runs
LAST_RESULTS = {}           # debug: per-core raw results


def _wrap_idx(idx):
    """int16 gather index layout: [128, len/16], i -> [i%16, i//16], tiled x8."""
    n = idx.shape[0]
    assert n % 16 == 0
    w = idx.reshape(n // 16, 16).T.astype(np.int16)   # [16, n/16]
    return np.tile(w, (8, 1))                          # [128, n/16]


def _preprocess(edge_index):
    """Partition/sort/pad edges (incl. self-loops) and build per-core
    gather-index and weighted-one-hot slabs."""
    src0 = np.asarray(edge_index[0], dtype=np.int64)
    dst0 = np.asarray(edge_index[1], dtype=np.int64)

    deg = np.bincount(dst0, minlength=N).astype(np.float64) + 1.0
    dinv = 1.0 / np.sqrt(deg)

    # self-loop contribution xw * dinv^2 == edge (i, i) with the same
    # dinv_src * dinv_dst weight formula
    src_a = np.concatenate([src0, np.arange(N, dtype=np.int64)])
    dst_a = np.concatenate([dst0, np.arange(N, dtype=np.int64)])
    w_a = (dinv[src_a] * dinv[dst_a]).astype(np.float32)

    owner = dst_a // NP
    wwin = (dst_a - owner * NP) // P
    slot_a = (dst_a - owner * NP - wwin * P).astype(np.int64)
    ch_a = src_a // CHROWS
    flat = ((owner * NW + wwin) * NCH + ch_a).astype(np.int64)
    cnt = np.bincount(flat, minlength=NCORES * NW * NCH).reshape(NCORES, NW, NCH)
    ngrp = np.ceil(cnt.max(axis=0) / P).astype(np.int64)  # [NW, NCH]
    ngrp_flat = np.concatenate([ngrp[w] for w in range(NW)])  # (w, ch) order
    NG = int(ngrp.sum())

    # sort: bucket-major, then src within bucket (HBM locality for gathers)
    order_all = np.lexsort((src_a, flat))
    bounds = np.searchsorted(flat[order_all], np.arange(NCORES * NW * NCH + 1))

    # padded stream layout per core, (w, ch) bucket order; bucket (w, ch)
    # occupies ngrp[w, ch]*P slots. Stream offsets per bucket:
    pad_len = ngrp * P                         # [NW, NCH]
    # per-chunk stream lengths (for gather calls): L[ch] = sum_w ngrp[w,ch]*P
    L = [int(ngrp[:, ch].sum()) * P for ch in range(NCH)]

    cores = []
    for c in range(NCORES):
        # per-chunk local row ids in (w, g) order; flat group order (w, ch, g)
        idx_parts = [[] for _ in range(NCH)]
        # flat padded stream (group-major) for one-hot construction
        slot_stream = np.zeros(NG * P, dtype=np.int64)
        wgt_stream = np.zeros(NG * P, dtype=np.float32)
        goff = 0
        for w in range(NW):
            for ch in range(NCH):
                b = (c * NW + w) * NCH + ch
                ee = order_all[bounds[b]:bounds[b + 1]]
                k = ee.shape[0]
                npadded = int(ngrp[w, ch]) * P
                loc = np.zeros(npadded, dtype=np.int64)
                loc[:k] = src_a[ee] - ch * CHROWS
                idx_parts[ch].append(loc)
                slot_stream[goff:goff + k] = slot_a[ee]
                wgt_stream[goff:goff + k] = w_a[ee]
                goff += npadded
        widx = np.concatenate(
            [_wrap_idx(np.concatenate(p)) if p else np.zeros((128, 0), np.int16)
             for p in idx_parts], axis=1)
        # weighted one-hot slab [P, NG*P]: position i of group g ->
        # partition i%P is wrong -- stream is group-major so position
        # pos = g*P + i, partition = i, column = g*P + slot
        pos = np.arange(NG * P)
        part = pos % P
        grp = pos // P
        ohw = np.zeros((P, NG * P), dtype=BF16)
        vals = wgt_stream.astype(BF16)
        ohw[part, grp * P + slot_stream] = vals
        cores.append((widx, ohw))
    return ngrp, L, NG, dinv.astype(np.float32), cores


def _build(ngrp, L, NG, has_bias, has_bias2, has_blin):
    """Build the SPMD bass program (same for all cores)."""
    nc = bacc.Bacc("TRN2", num_devices=NCORES)
    f32 = mybir.dt.float32
    bf16 = mybir.dt.bfloat16

    Loff = np.concatenate([[0], np.cumsum(L)])
    Ltot16 = int(Loff[-1]) // 16

    # ---- I/O ----
    xT = nc.dram_tensor("xT", [P, NPAD], bf16, kind="ExternalInput")
    widx_d = nc.dram_tensor("widx", [P, Ltot16], mybir.dt.int16,
                            kind="ExternalInput")
    ohw_d = nc.dram_tensor("ohw", [P, NG * P], bf16, kind="ExternalInput")
    ohb_d = nc.dram_tensor("ohb", [P, NW * P], bf16, kind="ExternalInput")
    cntinvb_d = nc.dram_tensor("cntinvb", [P, G], f32, kind="ExternalInput")
    Ws_d = [nc.dram_tensor(f"W{i}", [P, H], bf16, kind="ExternalInput")
            for i in range(3)]
    Wlin_d = nc.dram_tensor("Wlin", [P, C], f32, kind="ExternalInput")
    biasT_d = b2bc_d = blinb_d = None
    if has_bias:
        biasT_d = nc.dram_tensor("biasT", [P, 3], f32, kind="ExternalInput")
    if has_bias2:
        b2bc_d = nc.dram_tensor("b2bc", [P, H], f32, kind="ExternalInput")
    if has_blin:
        blinb_d = nc.dram_tensor("blinb", [P, C], f32, kind="ExternalInput")
    out_d = nc.dram_tensor("out", [G, C], f32, kind="ExternalOutput")

    y_local = nc.dram_tensor("y_local", [NP, H], bf16, kind="Internal")
    y_full = nc.dram_tensor("y_full", [N, H], bf16, kind="Internal",
                            addr_space="Shared")
    ar_in = nc.dram_tensor("ar_in", [P, G], f32, kind="Internal")
    ar_out = nc.dram_tensor("ar_out", [P, G], f32, kind="Internal",
                            addr_space="Shared")

    relu = mybir.ActivationFunctionType.Relu
    copyf = mybir.ActivationFunctionType.Copy

    with TileContext(nc) as tc:
        with ExitStack() as ctx:
            pers = ctx.enter_context(tc.tile_pool(name="pers", bufs=1))
            sy = ctx.enter_context(tc.tile_pool(name="sy", bufs=3))
            soh = ctx.enter_context(tc.tile_pool(name="soh", bufs=3))
            sep = ctx.enter_context(tc.tile_pool(name="sep", bufs=3))
            gpools = [ctx.enter_context(tc.tile_pool(name=f"gat{ch}", bufs=3))
                      for ch in range(NCH)]
            psy = ctx.enter_context(tc.tile_pool(name="psy", bufs=2, space="PSUM"))
            psa = ctx.enter_context(tc.tile_pool(name="psa", bufs=3, space="PSUM"))
            psp = ctx.enter_context(tc.tile_pool(name="psp", bufs=1, space="PSUM"))

            # ---- persistent tiles ----
            hT = pers.tile([P, NPAD], bf16)
            nc.sync.dma_start(out=hT[:], in_=xT[:])
            widx = pers.tile([P, Ltot16], mybir.dt.int16)
            nc.sync.dma_start(out=widx[:], in_=widx_d[:])
            ohb = pers.tile([P, NW * P], bf16)
            nc.sync.dma_start(out=ohb[:], in_=ohb_d[:])
            cntinvb = pers.tile([P, G], f32)
            nc.sync.dma_start(out=cntinvb[:], in_=cntinvb_d[:])
            Ws = []
            for i in range(3):
                t = pers.tile([P, H], bf16, tag=f"W{i}")
                nc.sync.dma_start(out=t[:], in_=Ws_d[i][:])
                Ws.append(t)
            Wlin = pers.tile([P, C], f32)
            nc.sync.dma_start(out=Wlin[:], in_=Wlin_d[:])
            biasT = b2bc = blinb = None
            if has_bias:
                biasT = pers.tile([P, 3], f32)
                nc.sync.dma_start(out=biasT[:], in_=biasT_d[:])
            if has_bias2:
                b2bc = pers.tile([P, H], f32)
                nc.sync.dma_start(out=b2bc[:], in_=b2bc_d[:])
            if has_blin:
                blinb = pers.tile([P, C], f32)
                nc.sync.dma_start(out=blinb[:], in_=blinb_d[:])

            # gather call schedule per chunk: list of (start, n) within chunk
            calls = []
            for ch in range(NCH):
                cs = []
                p = 0
                while p < L[ch]:
                    n = min(NI, L[ch] - p)
                    cs.append((p, n))
                    p += n
                calls.append(cs)

            for layer in range(NLAYERS):
                # ---- y = h @ W  -> y_local -> allgather ----
                for w in range(NW):
                    py = psy.tile([P, H], f32, space="PSUM", tag="py")
                    nc.tensor.matmul(out=py[:], lhsT=hT[:, w * P:(w + 1) * P],
                                     rhs=Ws[layer][:], start=True, stop=True)
                    yt = sy.tile([P, H], bf16, tag="yt")
                    nc.scalar.activation(out=yt[:], in_=py[:], func=copyf)
                    rows = min(NP - w * P, P)
                    nc.sync.dma_start(out=y_local[w * P:w * P + rows, :],
                                      in_=yt[:rows, :])
                nc.gpsimd.collective_compute(
                    "AllGather", mybir.AluOpType.bypass,
                    ins=[y_local[:]], outs=[y_full[:]],
                    replica_groups=[list(range(NCORES))],
                )

                # ---- edge gather + one-hot stream + segment-sum matmuls ----
                cur = [-1] * NCH          # current gather call per chunk
                gtile = [None] * NCH
                pos = [0] * NCH           # consumed rows within chunk stream

                def next_group(ch):
                    if cur[ch] < 0 or pos[ch] >= calls[ch][cur[ch]][0] + calls[ch][cur[ch]][1]:
                        cur[ch] += 1
                        start, n = calls[ch][cur[ch]]
                        c0 = (Loff[ch] + start) // 16
                        gt = gpools[ch].tile([P, NI // P, H], bf16, tag=f"g{ch}")
                        rows0 = ch * CHROWS
                        rows1 = min(rows0 + CHROWS, N)
                        nc.gpsimd.dma_gather(
                            out_ap=gt[:, :n // P, :],
                            in_ap=y_full[rows0:rows1],
                            idxs_ap=widx[:, c0:c0 + n // 16],
                            num_idxs=n,
                            num_idxs_reg=n,
                            elem_size=H,
                        )
                        gtile[ch] = gt
                    start, _ = calls[ch][cur[ch]]
                    t = (pos[ch] - start) // P
                    pos[ch] += P
                    return gtile[ch][:, t, :]

                ohw_cur = [-1]
                ohw_tile = [None]

                def next_ohw(gcol):
                    blk = gcol // OHK
                    if blk != ohw_cur[0]:
                        ohw_cur[0] = blk
                        c0 = blk * OHK * P
                        w_cols = min(OHK * P, NG * P - c0)
                        t = soh.tile([P, OHK * P], bf16, tag="oh")
                        nc.sync.dma_start(out=t[:, :w_cols],
                                          in_=ohw_d[:, c0:c0 + w_cols])
                        ohw_tile[0] = t
                    o = gcol % OHK
                    return ohw_tile[0][:, o * P:(o + 1) * P]

                gcol = 0
                pool_ps = None
                if layer == NLAYERS - 1:
                    pool_ps = psp.tile([P, G], f32, space="PSUM", tag="pp")
                for w in range(NW):
                    pa = psa.tile([P, P], f32, space="PSUM", tag="pa")
                    ng_w = int(ngrp[w].sum())
                    done = 0
                    for ch in range(NCH):
                        for g in range(int(ngrp[w, ch])):
                            ye = next_group(ch)
                            oh = next_ohw(gcol)
                            gcol += 1
                            done += 1
                            if layer < 2:
                                # reversed: out[f, slot]
                                nc.tensor.matmul(
                                    out=pa[:], lhsT=ye, rhs=oh,
                                    start=(done == 1), stop=(done == ng_w),
                                    skip_group_check=True)
                            else:
                                # forward: out[slot, f]
                                nc.tensor.matmul(
                                    out=pa[:], lhsT=oh, rhs=ye,
                                    start=(done == 1), stop=(done == ng_w),
                                    skip_group_check=True)

                    if layer < 2:
                        # epilogue on Scalar: relu(pa + b) -> hT window (bf16)
                        if has_bias:
                            nc.scalar.activation(
                                out=hT[:, w * P:(w + 1) * P], in_=pa[:],
                                func=relu, bias=biasT[:, layer:layer + 1])
                        else:
                            nc.scalar.activation(
                                out=hT[:, w * P:(w + 1) * P], in_=pa[:],
                                func=relu)
                    else:
                        h3 = sep.tile([P, H], bf16, tag="h3")
                        if has_bias2:
                            hb = sep.tile([P, H], f32, tag="hb")
                            nc.vector.tensor_tensor(
                                out=hb[:], in0=pa[:], in1=b2bc[:],
                                op=mybir.AluOpType.add)
                            nc.scalar.activation(out=h3[:], in_=hb[:], func=relu)
                        else:
                            nc.scalar.activation(out=h3[:], in_=pa[:], func=relu)
                        # pooling: poolT[f, g] += h3[n, f]^T @ ohb_w[n, g]
                        nc.tensor.matmul(
                            out=pool_ps[:], lhsT=h3[:],
                            rhs=ohb[:, w * P:(w + 1) * P],
                            start=(w == 0), stop=(w == NW - 1),
                            skip_group_check=True)

            # ---- pooling finish ----
            poolsb = sep.tile([P, G], f32, tag="poolsb")
            nc.vector.tensor_copy(out=poolsb[:], in_=pool_ps[:])
            nc.sync.dma_start(out=ar_in[:], in_=poolsb[:])
            nc.gpsimd.collective_compute(
                "AllReduce", mybir.AluOpType.add,
                ins=[ar_in[:]], outs=[ar_out[:]],
                replica_groups=[list(range(NCORES))],
            )
            art = sep.tile([P, G], f32, tag="art")
            nc.sync.dma_start(out=art[:], in_=ar_out[:])
            ptile = sep.tile([P, G], f32, tag="ptile")
            nc.vector.tensor_tensor(out=ptile[:], in0=art[:], in1=cntinvb[:],
                                    op=mybir.AluOpType.mult)
            po = psy.tile([P, C], f32, space="PSUM", tag="po")
            nc.tensor.matmul(out=po[:], lhsT=ptile[:], rhs=Wlin[:],
                             start=True, stop=True)
            ot = sep.tile([P, C], f32, tag="ot")
            if has_blin:
                nc.vector.tensor_tensor(out=ot[:], in0=po[:], in1=blinb[:],
                                        op=mybir.AluOpType.add)
            else:
                nc.vector.tensor_copy(out=ot[:], in_=po[:])
            nc.sync.dma_start(out=out_d[:], in_=ot[:G, :])

    nc.compile()
    return nc


def kernel(x, edge_index, batch, W0, b0, W1, b1, W2, b2, Wlin, blin):
    x = np.asarray(x, dtype=np.float32)
    batch_np = np.asarray(batch, dtype=np.int64)
    Wl = [np.asarray(w, dtype=np.float32) for w in (W0, W1, W2)]
    bl = [np.asarray(b, dtype=np.float32) for b in (b0, b1, b2)]
    Wlin = np.asarray(Wlin, dtype=np.float32)
    blin = np.asarray(blin, dtype=np.float32)

    ngrp, L, NG, dinv, cores = _preprocess(np.asarray(edge_index))
    has_bias = any(np.abs(b).max() > 0 for b in bl[:2])
    has_bias2 = bool(np.abs(bl[2]).max() > 0)
    has_blin = bool(np.abs(blin).max() > 0)

    cnt = np.bincount(batch_np, minlength=G).astype(np.float32)
    cntinv = (1.0 / np.maximum(cnt, 1.0)).astype(np.float32)
    cntinvb = np.tile(cntinv[None, :], (P, 1)).astype(np.float32)  # [P, G]

    in_maps = []
    for c in range(NCORES):
        widx, ohw = cores[c]
        lo = c * NP
        xTa = np.zeros((P, NPAD), dtype=BF16)
        xTa[:, :NP] = x[lo:lo + NP].T.astype(BF16)
        # batch one-hot slab [P, NW*P]: ohb[i, w*P + g] = (batch[lo+w*P+i]==g)
        ohb = np.zeros((P, NW * P), dtype=BF16)
        pos = np.arange(NP)
        wv = pos // P
        iv = pos % P
        ohb[iv, wv * P + batch_np[lo:lo + NP]] = BF16(1.0)
        m = {
            "xT": xTa, "widx": widx, "ohw": ohw, "ohb": ohb,
            "cntinvb": cntinvb,
            "W0": Wl[0].astype(BF16), "W1": Wl[1].astype(BF16),
            "W2": Wl[2].astype(BF16), "Wlin": Wlin,
        }
        if has_bias:
            m["biasT"] = np.stack([bl[0], bl[1], np.zeros(H, np.float32)],
                                  axis=1).astype(np.float32)
        if has_bias2:
            m["b2bc"] = np.tile(bl[2][None, :], (P, 1)).astype(np.float32)
        if has_blin:
            m["blinb"] = np.tile(blin[None, :], (P, 1)).astype(np.float32)
        in_maps.append(m)

    nc = _build(ngrp, L, NG, has_bias, has_bias2, has_blin)
    res = run_bass_kernel_spmd(nc, in_maps, core_ids=list(range(NCORES)),
                               trace=TRACE)
    global LAST_RESULTS
    LAST_RESULTS = res
    return res.results[0]["out"]


# revision 5
# speedup vs baseline: 1.8106x; 1.8106x over previous
"""Distributed GCN (3x GCNConv + global mean pool + linear) on 8 TRN2 cores.

Sharding: nodes partitioned contiguously across 8 cores; edges partitioned by
dst owner; per-layer node features (bf16) all-gathered to a replicated table
in each core's DRAM; per-edge features fetched with dma_gather (SWDGE);
segment-sum over dst done with precomputed *weighted* one-hot tiles (edge
norm dinv_src*dinv_dst baked in, self-loops appended as edges) streamed from
DRAM via HWDGE.  Layers 1-2 use the reversed matmul form (out [feat, node])
so the aggregation output directly feeds the next layer's lhsT; layer 3 uses
the forward form so pooling can consume [node, feat] tiles via a single
PSUM-accumulated one-hot matmul chain.  Epilogues run on the Scalar engine
(PSUM -> SBUF relu + cast).  The Vector engine is idle in steady state, so
SWDGE descriptor generation never contends for the shared SBUF port pair.
"""
import math
import numpy as np
import ml_dtypes
from contextlib import ExitStack

import concourse.bacc as bacc
import concourse.mybir as mybir
from concourse.tile import TileContext
from concourse.bass_utils import run_bass_kernel_spmd

P = 128
NCORES = 8
N = 100000
E = 1600000
H = 128
C = 10
G = 128
NP = N // NCORES            # 12500 nodes per core
NW = math.ceil(NP / P)      # 98 dst windows per core
NPAD = NW * P               # 12544 padded nodes per core
CHROWS = 25000              # gather chunk rows (int16 idx limit 32767)
NCH = math.ceil(N / CHROWS)  # 4
NI = 1024                   # indices per dma_gather call
OHK = 32                    # one-hot groups fetched per DMA
NLAYERS = 3                 # debug knob

BF16 = ml_dtypes.bfloat16

TRACE = False               # set by test.py for profiling runs
LAST_RESULTS = {}           # debug: per-core raw results


def _wrap_idx(idx):
    """int16 gather index layout: [128, len/16], i -> [i%16, i//16], tiled x8."""
    n = idx.shape[0]
    assert n % 16 == 0
    w = idx.reshape(n // 16, 16).T.astype(np.int16)   # [16, n/16]
    return np.tile(w, (8, 1))                          # [128, n/16]


def _preprocess(edge_index):
    """Partition/sort/pad edges (incl. self-loops) and build per-core
    gather-index and weighted-one-hot slabs."""
    src0 = np.asarray(edge_index[0], dtype=np.int64)
    dst0 = np.asarray(edge_index[1], dtype=np.int64)

    deg = np.bincount(dst0, minlength=N).astype(np.float64) + 1.0
    dinv = 1.0 / np.sqrt(deg)

    # self-loop contribution xw * dinv^2 == edge (i, i) with the same
    # dinv_src * dinv_dst weight formula
    src_a = np.concatenate([src0, np.arange(N, dtype=np.int64)])
    dst_a = np.concatenate([dst0, np.arange(N, dtype=np.int64)])
    w_a = (dinv[src_a] * dinv[dst_a]).astype(np.float32)

    owner = dst_a // NP
    wwin = (dst_a - owner * NP) // P
    slot_a = (dst_a - owner * NP - wwin * P).astype(np.int64)
    ch_a = src_a // CHROWS
    flat = ((owner * NW + wwin) * NCH + ch_a).astype(np.int64)
    cnt = np.bincount(flat, minlength=NCORES * NW * NCH).reshape(NCORES, NW, NCH)
    ngrp = np.ceil(cnt.max(axis=0) / P).astype(np.int64)  # [NW, NCH]
    ngrp_flat = np.concatenate([ngrp[w] for w in range(NW)])  # (w, ch) order
    NG = int(ngrp.sum())

    # sort: bucket-major, then src within bucket (HBM locality for gathers)
    order_all = np.lexsort((src_a, flat))
    bounds = np.searchsorted(flat[order_all], np.arange(NCORES * NW * NCH + 1))

    # padded stream layout per core, (w, ch) bucket order; bucket (w, ch)
    # occupies ngrp[w, ch]*P slots. Stream offsets per bucket:
    pad_len = ngrp * P                         # [NW, NCH]
    # per-chunk stream lengths (for gather calls): L[ch] = sum_w ngrp[w,ch]*P
    L = [int(ngrp[:, ch].sum()) * P for ch in range(NCH)]

    cores = []
    for c in range(NCORES):
        # per-chunk local row ids in (w, g) order; flat group order (w, ch, g)
        idx_parts = [[] for _ in range(NCH)]
        # flat padded stream (group-major) for one-hot construction
        slot_stream = np.zeros(NG * P, dtype=np.int64)
        wgt_stream = np.zeros(NG * P, dtype=np.float32)
        goff = 0
        for w in range(NW):
            for ch in range(NCH):
                b = (c * NW + w) * NCH + ch
                ee = order_all[bounds[b]:bounds[b + 1]]
                k = ee.shape[0]
                npadded = int(ngrp[w, ch]) * P
                loc = np.zeros(npadded, dtype=np.int64)
                loc[:k] = src_a[ee] - ch * CHROWS
                idx_parts[ch].append(loc)
                slot_stream[goff:goff + k] = slot_a[ee]
                wgt_stream[goff:goff + k] = w_a[ee]
                goff += npadded
        widx = np.concatenate(
            [_wrap_idx(np.concatenate(p)) if p else np.zeros((128, 0), np.int16)
             for p in idx_parts], axis=1)
        # weighted one-hot slab [P, NG*P]: position i of group g ->
        # partition i%P is wrong -- stream is group-major so position
        # pos = g*P + i, partition = i, column = g*P + slot
        pos = np.arange(NG * P)
        part = pos % P
        grp = pos // P
        ohw = np.zeros((P, NG * P), dtype=BF16)
        vals = wgt_stream.astype(BF16)
        ohw[part, grp * P + slot_stream] = vals
        cores.append((widx, ohw))
    return ngrp, L, NG, dinv.astype(np.float32), cores


def _build(ngrp, L, NG, has_bias, has_bias2, has_blin):
    """Build the SPMD bass program (same for all cores)."""
    nc = bacc.Bacc("TRN2", num_devices=NCORES, num_swdge_queues=4,
                   dynamic_dma_scratch_size=32768)
    f32 = mybir.dt.float32
    bf16 = mybir.dt.bfloat16

    Loff = np.concatenate([[0], np.cumsum(L)])
    Ltot16 = int(Loff[-1]) // 16

    # ---- I/O ----
    xT = nc.dram_tensor("xT", [P, NPAD], bf16, kind="ExternalInput")
    widx_d = nc.dram_tensor("widx", [P, Ltot16], mybir.dt.int16,
                            kind="ExternalInput")
    ohw_d = nc.dram_tensor("ohw", [P, NG * P], bf16, kind="ExternalInput")
    ohb_d = nc.dram_tensor("ohb", [P, NW * P], bf16, kind="ExternalInput")
    cntinvb_d = nc.dram_tensor("cntinvb", [P, G], f32, kind="ExternalInput")
    Ws_d = [nc.dram_tensor(f"W{i}", [P, H], bf16, kind="ExternalInput")
            for i in range(3)]
    Wlin_d = nc.dram_tensor("Wlin", [P, C], f32, kind="ExternalInput")
    biasT_d = b2bc_d = blinb_d = None
    if has_bias:
        biasT_d = nc.dram_tensor("biasT", [P, 3], f32, kind="ExternalInput")
    if has_bias2:
        b2bc_d = nc.dram_tensor("b2bc", [P, H], f32, kind="ExternalInput")
    if has_blin:
        blinb_d = nc.dram_tensor("blinb", [P, C], f32, kind="ExternalInput")
    out_d = nc.dram_tensor("out", [G, C], f32, kind="ExternalOutput")

    y_local = nc.dram_tensor("y_local", [NP, H], bf16, kind="Internal")
    y_full = nc.dram_tensor("y_full", [N, H], bf16, kind="Internal",
                            addr_space="Shared")
    ar_in = nc.dram_tensor("ar_in", [P, G], f32, kind="Internal")
    ar_out = nc.dram_tensor("ar_out", [P, G], f32, kind="Internal",
                            addr_space="Shared")

    relu = mybir.ActivationFunctionType.Relu
    copyf = mybir.ActivationFunctionType.Copy

    with TileContext(nc) as tc:
        with ExitStack() as ctx:
            pers = ctx.enter_context(tc.tile_pool(name="pers", bufs=1))
            sy = ctx.enter_context(tc.tile_pool(name="sy", bufs=3))
            soh = ctx.enter_context(tc.tile_pool(name="soh", bufs=3))
            sep = ctx.enter_context(tc.tile_pool(name="sep", bufs=3))
            gpools = [ctx.enter_context(tc.tile_pool(name=f"gat{ch}", bufs=3))
                      for ch in range(NCH)]
            psy = ctx.enter_context(tc.tile_pool(name="psy", bufs=2, space="PSUM"))
            psa = ctx.enter_context(tc.tile_pool(name="psa", bufs=3, space="PSUM"))
            psp = ctx.enter_context(tc.tile_pool(name="psp", bufs=1, space="PSUM"))

            # ---- persistent tiles ----
            hT = pers.tile([P, NPAD], bf16)
            nc.sync.dma_start(out=hT[:], in_=xT[:])
            widx = pers.tile([P, Ltot16], mybir.dt.int16)
            nc.sync.dma_start(out=widx[:], in_=widx_d[:])
            ohb = pers.tile([P, NW * P], bf16)
            nc.sync.dma_start(out=ohb[:], in_=ohb_d[:])
            cntinvb = pers.tile([P, G], f32)
            nc.sync.dma_start(out=cntinvb[:], in_=cntinvb_d[:])
            Ws = []
            for i in range(3):
                t = pers.tile([P, H], bf16, tag=f"W{i}")
                nc.sync.dma_start(out=t[:], in_=Ws_d[i][:])
                Ws.append(t)
            Wlin = pers.tile([P, C], f32)
            nc.sync.dma_start(out=Wlin[:], in_=Wlin_d[:])
            biasT = b2bc = blinb = None
            if has_bias:
                biasT = pers.tile([P, 3], f32)
                nc.sync.dma_start(out=biasT[:], in_=biasT_d[:])
            if has_bias2:
                b2bc = pers.tile([P, H], f32)
                nc.sync.dma_start(out=b2bc[:], in_=b2bc_d[:])
            if has_blin:
                blinb = pers.tile([P, C], f32)
                nc.sync.dma_start(out=blinb[:], in_=blinb_d[:])

            # gather call schedule per chunk: list of (start, n) within chunk
            calls = []
            for ch in range(NCH):
                cs = []
                p = 0
                while p < L[ch]:
                    n = min(NI, L[ch] - p)
                    cs.append((p, n))
                    p += n
                calls.append(cs)

            for layer in range(NLAYERS):
                # ---- y = h @ W  -> y_local -> allgather ----
                for w in range(NW):
                    py = psy.tile([P, H], f32, space="PSUM", tag="py")
                    nc.tensor.matmul(out=py[:], lhsT=hT[:, w * P:(w + 1) * P],
                                     rhs=Ws[layer][:], start=True, stop=True)
                    yt = sy.tile([P, H], bf16, tag="yt")
                    nc.scalar.activation(out=yt[:], in_=py[:], func=copyf)
                    rows = min(NP - w * P, P)
                    nc.sync.dma_start(out=y_local[w * P:w * P + rows, :],
                                      in_=yt[:rows, :])
                nc.gpsimd.collective_compute(
                    "AllGather", mybir.AluOpType.bypass,
                    ins=[y_local[:]], outs=[y_full[:]],
                    replica_groups=[list(range(NCORES))],
                )

                # ---- edge gather + one-hot stream + segment-sum matmuls ----
                cur = [-1] * NCH          # current gather call per chunk
                gtile = [None] * NCH
                pos = [0] * NCH           # consumed rows within chunk stream

                def next_group(ch):
                    if cur[ch] < 0 or pos[ch] >= calls[ch][cur[ch]][0] + calls[ch][cur[ch]][1]:
                        cur[ch] += 1
                        start, n = calls[ch][cur[ch]]
                        c0 = (Loff[ch] + start) // 16
                        gt = gpools[ch].tile([P, NI // P, H], bf16, tag=f"g{ch}")
                        rows0 = ch * CHROWS
                        rows1 = min(rows0 + CHROWS, N)
                        nc.gpsimd.dma_gather(
                            out_ap=gt[:, :n // P, :],
                            in_ap=y_full[rows0:rows1],
                            idxs_ap=widx[:, c0:c0 + n // 16],
                            num_idxs=n,
                            num_idxs_reg=n,
                            elem_size=H,
                            single_packet=False,
                            queue_num=ch,
                        )
                        gtile[ch] = gt
                    start, _ = calls[ch][cur[ch]]
                    t = (pos[ch] - start) // P
                    pos[ch] += P
                    return gtile[ch][:, t, :]

                ohw_cur = [-1]
                ohw_tile = [None]

                def next_ohw(gcol):
                    blk = gcol // OHK
                    if blk != ohw_cur[0]:
                        ohw_cur[0] = blk
                        c0 = blk * OHK * P
                        w_cols = min(OHK * P, NG * P - c0)
                        t = soh.tile([P, OHK * P], bf16, tag="oh")
                        nc.sync.dma_start(out=t[:, :w_cols],
                                          in_=ohw_d[:, c0:c0 + w_cols])
                        ohw_tile[0] = t
                    o = gcol % OHK
                    return ohw_tile[0][:, o * P:(o + 1) * P]

                gcol = 0
                pool_ps = None
                if layer == NLAYERS - 1:
                    pool_ps = psp.tile([P, G], f32, space="PSUM", tag="pp")
                for w in range(NW):
                    pa = psa.tile([P, P], f32, space="PSUM", tag="pa")
                    ng_w = int(ngrp[w].sum())
                    done = 0
                    for ch in range(NCH):
                        for g in range(int(ngrp[w, ch])):
                            ye = next_group(ch)
                            oh = next_ohw(gcol)
                            gcol += 1
                            done += 1
                            if layer < 2:
                                # reversed: out[f, slot]
                                nc.tensor.matmul(
                                    out=pa[:], lhsT=ye, rhs=oh,
                                    start=(done == 1), stop=(done == ng_w),
                                    skip_group_check=True)
                            else:
                                # forward: out[slot, f]
                                nc.tensor.matmul(
                                    out=pa[:], lhsT=oh, rhs=ye,
                                    start=(done == 1), stop=(done == ng_w),
                                    skip_group_check=True)

                    if layer < 2:
                        # epilogue on Scalar: relu(pa + b) -> hT window (bf16)
                        if has_bias:
                            nc.scalar.activation(
                                out=hT[:, w * P:(w + 1) * P], in_=pa[:],
                                func=relu, bias=biasT[:, layer:layer + 1])
                        else:
                            nc.scalar.activation(
                                out=hT[:, w * P:(w + 1) * P], in_=pa[:],
                                func=relu)
                    else:
                        h3 = sep.tile([P, H], bf16, tag="h3")
                        if has_bias2:
                            hb = sep.tile([P, H], f32, tag="hb")
                            nc.vector.tensor_tensor(
                                out=hb[:], in0=pa[:], in1=b2bc[:],
                                op=mybir.AluOpType.add)
                            nc.scalar.activation(out=h3[:], in_=hb[:], func=relu)
                        else:
                            nc.scalar.activation(out=h3[:], in_=pa[:], func=relu)
                        # pooling: poolT[f, g] += h3[n, f]^T @ ohb_w[n, g]
                        nc.tensor.matmul(
                            out=pool_ps[:], lhsT=h3[:],
                            rhs=ohb[:, w * P:(w + 1) * P],
                            start=(w == 0), stop=(w == NW - 1),
                            skip_group_check=True)

            # ---- pooling finish ----
            poolsb = sep.tile([P, G], f32, tag="poolsb")
            nc.vector.tensor_copy(out=poolsb[:], in_=pool_ps[:])
            nc.sync.dma_start(out=ar_in[:], in_=poolsb[:])
            nc.gpsimd.collective_compute(
                "AllReduce", mybir.AluOpType.add,
                ins=[ar_in[:]], outs=[ar_out[:]],
                replica_groups=[list(range(NCORES))],
            )
            art = sep.tile([P, G], f32, tag="art")
            nc.sync.dma_start(out=art[:], in_=ar_out[:])
            ptile = sep.tile([P, G], f32, tag="ptile")
            nc.vector.tensor_tensor(out=ptile[:], in0=art[:], in1=cntinvb[:],
                                    op=mybir.AluOpType.mult)
            po = psy.tile([P, C], f32, space="PSUM", tag="po")
            nc.tensor.matmul(out=po[:], lhsT=ptile[:], rhs=Wlin[:],
                             start=True, stop=True)
            ot = sep.tile([P, C], f32, tag="ot")
            if has_blin:
                nc.vector.tensor_tensor(out=ot[:], in0=po[:], in1=blinb[:],
                                        op=mybir.AluOpType.add)
            else:
                nc.vector.tensor_copy(out=ot[:], in_=po[:])
            nc.sync.dma_start(out=out_d[:], in_=ot[:G, :])

    nc.compile()
    return nc


def kernel(x, edge_index, batch, W0, b0, W1, b1, W2, b2, Wlin, blin):
    x = np.asarray(x, dtype=np.float32)
    batch_np = np.asarray(batch, dtype=np.int64)
    Wl = [np.asarray(w, dtype=np.float32) for w in (W0, W1, W2)]
    bl = [np.asarray(b, dtype=np.float32) for b in (b0, b1, b2)]
    Wlin = np.asarray(Wlin, dtype=np.float32)
    blin = np.asarray(blin, dtype=np.float32)

    ngrp, L, NG, dinv, cores = _preprocess(np.asarray(edge_index))
    has_bias = any(np.abs(b).max() > 0 for b in bl[:2])
    has_bias2 = bool(np.abs(bl[2]).max() > 0)
    has_blin = bool(np.abs(blin).max() > 0)

    cnt = np.bincount(batch_np, minlength=G).astype(np.float32)
    cntinv = (1.0 / np.maximum(cnt, 1.0)).astype(np.float32)
    cntinvb = np.tile(cntinv[None, :], (P, 1)).astype(np.float32)  # [P, G]

    in_maps = []
    for c in range(NCORES):
        widx, ohw = cores[c]
        lo = c * NP
        xTa = np.zeros((P, NPAD), dtype=BF16)
        xTa[:, :NP] = x[lo:lo + NP].T.astype(BF16)
        # batch one-hot slab [P, NW*P]: ohb[i, w*P + g] = (batch[lo+w*P+i]==g)
        ohb = np.zeros((P, NW * P), dtype=BF16)
        pos = np.arange(NP)
        wv = pos // P
        iv = pos % P
        ohb[iv, wv * P + batch_np[lo:lo + NP]] = BF16(1.0)
        m = {
            "xT": xTa, "widx": widx, "ohw": ohw, "ohb": ohb,
            "cntinvb": cntinvb,
            "W0": Wl[0].astype(BF16), "W1": Wl[1].astype(BF16),
            "W2": Wl[2].astype(BF16), "Wlin": Wlin,
        }
        if has_bias:
            m["biasT"] = np.stack([bl[0], bl[1], np.zeros(H, np.float32)],
                                  axis=1).astype(np.float32)
        if has_bias2:
            m["b2bc"] = np.tile(bl[2][None, :], (P, 1)).astype(np.float32)
        if has_blin:
            m["blinb"] = np.tile(blin[None, :], (P, 1)).astype(np.float32)
        in_maps.append(m)

    nc = _build(ngrp, L, NG, has_bias, has_bias2, has_blin)
    res = run_bass_kernel_spmd(nc, in_maps, core_ids=list(range(NCORES)),
                               trace=TRACE)
    global LAST_RESULTS
    LAST_RESULTS = res
    return res.results[0]["out"]


# revision 12
# speedup vs baseline: 2.1317x; 1.1774x over previous
"""Distributed GCN (3x GCNConv + global mean pool + linear) on 8 TRN2 cores.

Sharding: nodes partitioned contiguously across 8 cores; edges partitioned by
dst owner; per-layer node features (bf16) all-gathered to a replicated table
in each core's DRAM; per-edge features fetched with dma_gather (SWDGE);
segment-sum over dst done with precomputed *weighted* one-hot tiles (edge
norm dinv_src*dinv_dst baked in, self-loops appended as edges) streamed from
DRAM via HWDGE.  Layers 1-2 use the reversed matmul form (out [feat, node])
so the aggregation output directly feeds the next layer's lhsT; layer 3 uses
the forward form so pooling can consume [node, feat] tiles via a single
PSUM-accumulated one-hot matmul chain.  Epilogues run on the Scalar engine
(PSUM -> SBUF relu + cast).  The Vector engine is idle in steady state, so
SWDGE descriptor generation never contends for the shared SBUF port pair.
"""
import math
import numpy as np
import ml_dtypes
from contextlib import ExitStack

import concourse.bacc as bacc
import concourse.mybir as mybir
from concourse.tile import TileContext
from concourse.bass_utils import run_bass_kernel_spmd

P = 128
NCORES = 8
N = 100000
E = 1600000
H = 128
C = 10
G = 128
NP = N // NCORES            # 12500 nodes per core
NW = math.ceil(NP / P)      # 98 dst windows per core
NPAD = NW * P               # 12544 padded nodes per core
CHROWS = 25000              # gather chunk rows (int16 idx limit 32767)
NCH = math.ceil(N / CHROWS)  # 4
NI = 2048                   # indices per dma_gather call
OHK = 16                    # one-hot groups fetched per DMA
NLAYERS = 3                 # debug knob

BF16 = ml_dtypes.bfloat16

TRACE = False               # set by test.py for profiling runs
LAST_RESULTS = {}           # debug: per-core raw results


def _wrap_idx(idx):
    """int16 gather index layout: [128, len/16], i -> [i%16, i//16], tiled x8."""
    n = idx.shape[0]
    assert n % 16 == 0
    w = idx.reshape(n // 16, 16).T.astype(np.int16)   # [16, n/16]
    return np.tile(w, (8, 1))                          # [128, n/16]


def _preprocess(edge_index):
    """Partition/sort/pad edges (incl. self-loops) and build per-core
    gather-index and weighted-one-hot slabs."""
    src0 = np.asarray(edge_index[0], dtype=np.int64)
    dst0 = np.asarray(edge_index[1], dtype=np.int64)

    deg = np.bincount(dst0, minlength=N).astype(np.float64) + 1.0
    dinv = 1.0 / np.sqrt(deg)

    # self-loop term xw * dinv^2 is handled on-device (scaled-hT matmul),
    # not as gather edges
    src_a = src0
    dst_a = dst0
    w_a = (dinv[src_a] * dinv[dst_a]).astype(np.float32)

    owner = dst_a // NP
    wwin = (dst_a - owner * NP) // P
    slot_a = (dst_a - owner * NP - wwin * P).astype(np.int64)
    ch_a = src_a // CHROWS
    flat = ((owner * NW + wwin) * NCH + ch_a).astype(np.int64)
    cnt = np.bincount(flat, minlength=NCORES * NW * NCH).reshape(NCORES, NW, NCH)
    ngrp = np.ceil(cnt.max(axis=0) / P).astype(np.int64)  # [NW, NCH]
    ngrp_flat = np.concatenate([ngrp[w] for w in range(NW)])  # (w, ch) order
    NG = int(ngrp.sum())

    # sort: bucket-major, then src within bucket (HBM locality for gathers)
    order_all = np.lexsort((src_a, flat))
    bounds = np.searchsorted(flat[order_all], np.arange(NCORES * NW * NCH + 1))

    # padded stream layout per core, (w, ch) bucket order; bucket (w, ch)
    # occupies ngrp[w, ch]*P slots. Stream offsets per bucket:
    pad_len = ngrp * P                         # [NW, NCH]
    # per-chunk stream lengths (for gather calls): L[ch] = sum_w ngrp[w,ch]*P
    L = [int(ngrp[:, ch].sum()) * P for ch in range(NCH)]

    cores = []
    for c in range(NCORES):
        # per-chunk local row ids in (w, g) order; flat group order (w, ch, g)
        idx_parts = [[] for _ in range(NCH)]
        # flat padded stream (group-major) for one-hot construction
        slot_stream = np.zeros(NG * P, dtype=np.int64)
        wgt_stream = np.zeros(NG * P, dtype=np.float32)
        goff = 0
        for w in range(NW):
            for ch in range(NCH):
                b = (c * NW + w) * NCH + ch
                ee = order_all[bounds[b]:bounds[b + 1]]
                k = ee.shape[0]
                npadded = int(ngrp[w, ch]) * P
                loc = np.zeros(npadded, dtype=np.int64)
                loc[:k] = src_a[ee] - ch * CHROWS
                idx_parts[ch].append(loc)
                slot_stream[goff:goff + k] = slot_a[ee]
                wgt_stream[goff:goff + k] = w_a[ee]
                goff += npadded
        widx = np.concatenate(
            [_wrap_idx(np.concatenate(p)) if p else np.zeros((128, 0), np.int16)
             for p in idx_parts], axis=1)
        # weighted one-hot slab [P, NG*P]: position i of group g ->
        # partition i%P is wrong -- stream is group-major so position
        # pos = g*P + i, partition = i, column = g*P + slot
        pos = np.arange(NG * P)
        part = pos % P
        grp = pos // P
        ohw = np.zeros((P, NG * P), dtype=BF16)
        vals = wgt_stream.astype(BF16)
        ohw[part, grp * P + slot_stream] = vals
        cores.append((widx, ohw))
    return ngrp, L, NG, dinv.astype(np.float32), cores


def _build(ngrp, L, NG, has_bias, has_bias2, has_blin):
    """Build the SPMD bass program (same for all cores)."""
    nc = bacc.Bacc("TRN2", num_devices=NCORES, num_swdge_queues=4,
                   dynamic_dma_scratch_size=32768)
    f32 = mybir.dt.float32
    bf16 = mybir.dt.bfloat16

    Loff = np.concatenate([[0], np.cumsum(L)])
    Ltot16 = int(Loff[-1]) // 16

    # ---- I/O ----
    xT = nc.dram_tensor("xT", [P, NPAD], bf16, kind="ExternalInput")
    widx_d = nc.dram_tensor("widx", [P, Ltot16], mybir.dt.int16,
                            kind="ExternalInput")
    ohw_d = nc.dram_tensor("ohw", [P, NG * P], bf16, kind="ExternalInput")
    ohb_d = nc.dram_tensor("ohb", [P, NW * P], bf16, kind="ExternalInput")
    dinv2bc_d = nc.dram_tensor("dinv2bc", [P, NPAD], bf16, kind="ExternalInput")
    cntinvb_d = nc.dram_tensor("cntinvb", [P, G], f32, kind="ExternalInput")
    Ws_d = [nc.dram_tensor(f"W{i}", [P, H], bf16, kind="ExternalInput")
            for i in range(3)]
    Wlin_d = nc.dram_tensor("Wlin", [P, C], f32, kind="ExternalInput")
    biasT_d = b2bc_d = blinb_d = None
    if has_bias:
        biasT_d = nc.dram_tensor("biasT", [P, 3], f32, kind="ExternalInput")
    if has_bias2:
        b2bc_d = nc.dram_tensor("b2bc", [P, H], f32, kind="ExternalInput")
    if has_blin:
        blinb_d = nc.dram_tensor("blinb", [P, C], f32, kind="ExternalInput")
    out_d = nc.dram_tensor("out", [G, C], f32, kind="ExternalOutput")

    y_local = nc.dram_tensor("y_local", [NP, H], bf16, kind="Internal")
    y_full = nc.dram_tensor("y_full", [N, H], bf16, kind="Internal",
                            addr_space="Shared")
    ar_in = nc.dram_tensor("ar_in", [P, G], f32, kind="Internal")
    ar_out = nc.dram_tensor("ar_out", [P, G], f32, kind="Internal",
                            addr_space="Shared")

    relu = mybir.ActivationFunctionType.Relu
    copyf = mybir.ActivationFunctionType.Copy

    with TileContext(nc) as tc:
        with ExitStack() as ctx:
            pers = ctx.enter_context(tc.tile_pool(name="pers", bufs=1))
            sy = ctx.enter_context(tc.tile_pool(name="sy", bufs=3))
            soh = ctx.enter_context(tc.tile_pool(name="soh", bufs=3))
            sep = ctx.enter_context(tc.tile_pool(name="sep", bufs=3))
            gpools = [ctx.enter_context(tc.tile_pool(name=f"gat{ch}", bufs=2))
                      for ch in range(NCH)]
            psy = ctx.enter_context(tc.tile_pool(name="psy", bufs=2, space="PSUM"))
            psa = ctx.enter_context(tc.tile_pool(name="psa", bufs=3, space="PSUM"))
            psp = ctx.enter_context(tc.tile_pool(name="psp", bufs=1, space="PSUM"))

            # ---- persistent tiles ----
            hT = pers.tile([P, NPAD], bf16)
            nc.sync.dma_start(out=hT[:], in_=xT[:])
            widx = pers.tile([P, Ltot16], mybir.dt.int16)
            nc.sync.dma_start(out=widx[:], in_=widx_d[:])
            ohb = pers.tile([P, NW * P], bf16)
            nc.sync.dma_start(out=ohb[:], in_=ohb_d[:])
            dinv2bc = pers.tile([P, NPAD], bf16)
            nc.sync.dma_start(out=dinv2bc[:], in_=dinv2bc_d[:])
            cntinvb = pers.tile([P, G], f32)
            nc.sync.dma_start(out=cntinvb[:], in_=cntinvb_d[:])
            Ws = []
            for i in range(3):
                t = pers.tile([P, H], bf16, tag=f"W{i}")
                nc.sync.dma_start(out=t[:], in_=Ws_d[i][:])
                Ws.append(t)
            Wlin = pers.tile([P, C], f32)
            nc.sync.dma_start(out=Wlin[:], in_=Wlin_d[:])
            biasT = b2bc = blinb = None
            if has_bias:
                biasT = pers.tile([P, 3], f32)
                nc.sync.dma_start(out=biasT[:], in_=biasT_d[:])
            if has_bias2:
                b2bc = pers.tile([P, H], f32)
                nc.sync.dma_start(out=b2bc[:], in_=b2bc_d[:])
            if has_blin:
                blinb = pers.tile([P, C], f32)
                nc.sync.dma_start(out=blinb[:], in_=blinb_d[:])

            # gather call schedule per chunk: list of (start, n) within chunk
            calls = []
            for ch in range(NCH):
                cs = []
                p = 0
                while p < L[ch]:
                    n = min(NI, L[ch] - p)
                    cs.append((p, n))
                    p += n
                calls.append(cs)

            for layer in range(NLAYERS):
                # ---- y = h @ W  -> y_local -> allgather ----
                for w in range(NW):
                    py = psy.tile([P, H], f32, space="PSUM", tag="py")
                    nc.tensor.matmul(out=py[:], lhsT=hT[:, w * P:(w + 1) * P],
                                     rhs=Ws[layer][:], start=True, stop=True)
                    yt = sy.tile([P, H], bf16, tag="yt")
                    nc.scalar.activation(out=yt[:], in_=py[:], func=copyf)
                    rows = min(NP - w * P, P)
                    nc.sync.dma_start(out=y_local[w * P:w * P + rows, :],
                                      in_=yt[:rows, :])
                nc.gpsimd.collective_compute(
                    "AllGather", mybir.AluOpType.bypass,
                    ins=[y_local[:]], outs=[y_full[:]],
                    replica_groups=[list(range(NCORES))],
                )

                # ---- edge gather + one-hot stream + segment-sum matmuls ----
                cur = [-1] * NCH          # current gather call per chunk
                gtile = [None] * NCH
                pos = [0] * NCH           # consumed rows within chunk stream

                def next_group(ch):
                    if cur[ch] < 0 or pos[ch] >= calls[ch][cur[ch]][0] + calls[ch][cur[ch]][1]:
                        cur[ch] += 1
                        start, n = calls[ch][cur[ch]]
                        c0 = (Loff[ch] + start) // 16
                        gt = gpools[ch].tile([P, NI // P, H], bf16, tag=f"g{ch}")
                        rows0 = ch * CHROWS
                        rows1 = min(rows0 + CHROWS, N)
                        nc.gpsimd.dma_gather(
                            out_ap=gt[:, :n // P, :],
                            in_ap=y_full[rows0:rows1],
                            idxs_ap=widx[:, c0:c0 + n // 16],
                            num_idxs=n,
                            num_idxs_reg=n,
                            elem_size=H,
                            single_packet=False,
                            queue_num=ch,
                        )
                        gtile[ch] = gt
                    start, _ = calls[ch][cur[ch]]
                    t = (pos[ch] - start) // P
                    pos[ch] += P
                    return gtile[ch][:, t, :]

                ohw_cur = [-1]
                ohw_tile = [None]

                def next_ohw(gcol):
                    blk = gcol // OHK
                    if blk != ohw_cur[0]:
                        ohw_cur[0] = blk
                        c0 = blk * OHK * P
                        w_cols = min(OHK * P, NG * P - c0)
                        t = soh.tile([P, OHK * P], bf16, tag="oh")
                        nc.sync.dma_start(out=t[:, :w_cols],
                                          in_=ohw_d[:, c0:c0 + w_cols])
                        ohw_tile[0] = t
                    o = gcol % OHK
                    return ohw_tile[0][:, o * P:(o + 1) * P]

                gcol = 0
                pool_ps = None
                if layer == NLAYERS - 1:
                    pool_ps = psp.tile([P, G], f32, space="PSUM", tag="pp")
                for w in range(NW):
                    pa = psa.tile([P, P], f32, space="PSUM", tag="pa")
                    ng_w = int(ngrp[w].sum())
                    # self-loop term: (hT_w * dinv^2) @ W opens the PSUM chain
                    hts = sep.tile([P, P], bf16, tag="hts")
                    nc.vector.tensor_tensor(
                        out=hts[:], in0=hT[:, w * P:(w + 1) * P],
                        in1=dinv2bc[:, w * P:(w + 1) * P],
                        op=mybir.AluOpType.mult)
                    if layer < 2:
                        # reversed: out[f, slot] = W^T(fi,f) @ hts(fi, slot)
                        nc.tensor.matmul(
                            out=pa[:], lhsT=Ws[layer][:], rhs=hts[:],
                            start=True, stop=(ng_w == 0),
                            skip_group_check=True)
                    else:
                        # forward: out[slot, f] = hts^T(fi,slot) @ W(fi, f)
                        nc.tensor.matmul(
                            out=pa[:], lhsT=hts[:], rhs=Ws[layer][:],
                            start=True, stop=(ng_w == 0),
                            skip_group_check=True)
                    done = 0
                    for ch in range(NCH):
                        for g in range(int(ngrp[w, ch])):
                            ye = next_group(ch)
                            oh = next_ohw(gcol)
                            gcol += 1
                            done += 1
                            if layer < 2:
                                # reversed: out[f, slot]
                                nc.tensor.matmul(
                                    out=pa[:], lhsT=ye, rhs=oh,
                                    start=False, stop=(done == ng_w),
                                    skip_group_check=True)
                            else:
                                # forward: out[slot, f]
                                nc.tensor.matmul(
                                    out=pa[:], lhsT=oh, rhs=ye,
                                    start=False, stop=(done == ng_w),
                                    skip_group_check=True)

                    if layer < 2:
                        # epilogue on Scalar: relu(pa + b) -> hT window (bf16)
                        if has_bias:
                            nc.scalar.activation(
                                out=hT[:, w * P:(w + 1) * P], in_=pa[:],
                                func=relu, bias=biasT[:, layer:layer + 1])
                        else:
                            nc.scalar.activation(
                                out=hT[:, w * P:(w + 1) * P], in_=pa[:],
                                func=relu)
                    else:
                        h3 = sep.tile([P, H], bf16, tag="h3")
                        if has_bias2:
                            hb = sep.tile([P, H], f32, tag="hb")
                            nc.vector.tensor_tensor(
                                out=hb[:], in0=pa[:], in1=b2bc[:],
                                op=mybir.AluOpType.add)
                            nc.scalar.activation(out=h3[:], in_=hb[:], func=relu)
                        else:
                            nc.scalar.activation(out=h3[:], in_=pa[:], func=relu)
                        # pooling: poolT[f, g] += h3[n, f]^T @ ohb_w[n, g]
                        nc.tensor.matmul(
                            out=pool_ps[:], lhsT=h3[:],
                            rhs=ohb[:, w * P:(w + 1) * P],
                            start=(w == 0), stop=(w == NW - 1),
                            skip_group_check=True)

            # ---- pooling finish ----
            poolsb = sep.tile([P, G], f32, tag="poolsb")
            nc.vector.tensor_copy(out=poolsb[:], in_=pool_ps[:])
            nc.sync.dma_start(out=ar_in[:], in_=poolsb[:])
            nc.gpsimd.collective_compute(
                "AllReduce", mybir.AluOpType.add,
                ins=[ar_in[:]], outs=[ar_out[:]],
                replica_groups=[list(range(NCORES))],
            )
            art = sep.tile([P, G], f32, tag="art")
            nc.sync.dma_start(out=art[:], in_=ar_out[:])
            ptile = sep.tile([P, G], f32, tag="ptile")
            nc.vector.tensor_tensor(out=ptile[:], in0=art[:], in1=cntinvb[:],
                                    op=mybir.AluOpType.mult)
            po = psy.tile([P, C], f32, space="PSUM", tag="po")
            nc.tensor.matmul(out=po[:], lhsT=ptile[:], rhs=Wlin[:],
                             start=True, stop=True)
            ot = sep.tile([P, C], f32, tag="ot")
            if has_blin:
                nc.vector.tensor_tensor(out=ot[:], in0=po[:], in1=blinb[:],
                                        op=mybir.AluOpType.add)
            else:
                nc.vector.tensor_copy(out=ot[:], in_=po[:])
            nc.sync.dma_start(out=out_d[:], in_=ot[:G, :])

    nc.compile()
    return nc


def kernel(x, edge_index, batch, W0, b0, W1, b1, W2, b2, Wlin, blin):
    x = np.asarray(x, dtype=np.float32)
    batch_np = np.asarray(batch, dtype=np.int64)
    Wl = [np.asarray(w, dtype=np.float32) for w in (W0, W1, W2)]
    bl = [np.asarray(b, dtype=np.float32) for b in (b0, b1, b2)]
    Wlin = np.asarray(Wlin, dtype=np.float32)
    blin = np.asarray(blin, dtype=np.float32)

    ngrp, L, NG, dinv, cores = _preprocess(np.asarray(edge_index))
    has_bias = any(np.abs(b).max() > 0 for b in bl[:2])
    has_bias2 = bool(np.abs(bl[2]).max() > 0)
    has_blin = bool(np.abs(blin).max() > 0)

    cnt = np.bincount(batch_np, minlength=G).astype(np.float32)
    cntinv = (1.0 / np.maximum(cnt, 1.0)).astype(np.float32)
    cntinvb = np.tile(cntinv[None, :], (P, 1)).astype(np.float32)  # [P, G]

    in_maps = []
    for c in range(NCORES):
        widx, ohw = cores[c]
        lo = c * NP
        xTa = np.zeros((P, NPAD), dtype=BF16)
        xTa[:, :NP] = x[lo:lo + NP].T.astype(BF16)
        # batch one-hot slab [P, NW*P]: ohb[i, w*P + g] = (batch[lo+w*P+i]==g)
        ohb = np.zeros((P, NW * P), dtype=BF16)
        pos = np.arange(NP)
        wv = pos // P
        iv = pos % P
        ohb[iv, wv * P + batch_np[lo:lo + NP]] = BF16(1.0)
        d2 = np.zeros(NPAD, dtype=np.float32)
        d2[:NP] = dinv[lo:lo + NP] ** 2
        dinv2bc = np.tile(d2[None, :], (P, 1)).astype(BF16)
        m = {
            "xT": xTa, "widx": widx, "ohw": ohw, "ohb": ohb,
            "dinv2bc": dinv2bc, "cntinvb": cntinvb,
            "W0": Wl[0].astype(BF16), "W1": Wl[1].astype(BF16),
            "W2": Wl[2].astype(BF16), "Wlin": Wlin,
        }
        if has_bias:
            m["biasT"] = np.stack([bl[0], bl[1], np.zeros(H, np.float32)],
                                  axis=1).astype(np.float32)
        if has_bias2:
            m["b2bc"] = np.tile(bl[2][None, :], (P, 1)).astype(np.float32)
        if has_blin:
            m["blinb"] = np.tile(blin[None, :], (P, 1)).astype(np.float32)
        in_maps.append(m)

    nc = _build(ngrp, L, NG, has_bias, has_bias2, has_blin)
    res = run_bass_kernel_spmd(nc, in_maps, core_ids=list(range(NCORES)),
                               trace=TRACE)
    global LAST_RESULTS
    LAST_RESULTS = res
    return res.results[0]["out"]


# revision 22
# speedup vs baseline: 2.7959x; 1.3116x over previous
"""Distributed GCN (3x GCNConv + global mean pool + linear) on 8 TRN2 cores.

Sharding: nodes partitioned contiguously across 8 cores; edges partitioned by
dst owner; per-layer node features (bf16) all-gathered to a replicated table
in each core's DRAM; per-edge features fetched with dma_gather (SWDGE);
segment-sum over dst done with precomputed *weighted* one-hot tiles (edge
norm dinv_src*dinv_dst baked in, self-loops appended as edges) streamed from
DRAM via HWDGE.  Layers 1-2 use the reversed matmul form (out [feat, node])
so the aggregation output directly feeds the next layer's lhsT; layer 3 uses
the forward form so pooling can consume [node, feat] tiles via a single
PSUM-accumulated one-hot matmul chain.  Epilogues run on the Scalar engine
(PSUM -> SBUF relu + cast).  The Vector engine is idle in steady state, so
SWDGE descriptor generation never contends for the shared SBUF port pair.
"""
import math
import numpy as np
import ml_dtypes
from contextlib import ExitStack

import concourse.bacc as bacc
import concourse.mybir as mybir
from concourse.tile import TileContext
from concourse.bass_utils import run_bass_kernel_spmd

P = 128
NCORES = 8
N = 100000
E = 1600000
H = 128
C = 10
G = 128
NP = N // NCORES            # 12500 nodes per core
NW = math.ceil(NP / P)      # 98 dst windows per core
NPAD = NW * P               # 12544 padded nodes per core
CHROWS = 25000              # gather chunk rows (int16 idx limit 32767)
NCH = math.ceil(N / CHROWS)  # 4
NI = 2048                   # indices per dma_gather call
OHK = 16                    # one-hot groups fetched per DMA
NLAYERS = 3                 # debug knob

BF16 = ml_dtypes.bfloat16
FP8 = ml_dtypes.float8_e4m3

TRACE = False               # set by test.py for profiling runs
LAST_RESULTS = {}           # debug: per-core raw results


def _wrap_idx(idx):
    """int16 gather index layout: [128, len/16], i -> [i%16, i//16], tiled x8."""
    n = idx.shape[0]
    assert n % 16 == 0
    w = idx.reshape(n // 16, 16).T.astype(np.int16)   # [16, n/16]
    return np.tile(w, (8, 1))                          # [128, n/16]


def _preprocess(edge_index):
    """Partition/sort/pad edges (incl. self-loops) and build per-core
    gather-index and weighted-one-hot slabs."""
    src0 = np.asarray(edge_index[0], dtype=np.int64)
    dst0 = np.asarray(edge_index[1], dtype=np.int64)

    deg = np.bincount(dst0, minlength=N).astype(np.float64) + 1.0
    dinv = 1.0 / np.sqrt(deg)

    # self-loop term xw * dinv^2 is handled on-device (scaled-hT matmul),
    # not as gather edges
    src_a = src0
    dst_a = dst0
    w_a = (dinv[src_a] * dinv[dst_a]).astype(np.float32)

    owner = dst_a // NP
    wwin = (dst_a - owner * NP) // P
    slot_a = (dst_a - owner * NP - wwin * P).astype(np.int64)
    ch_a = src_a // CHROWS
    flat = ((owner * NW + wwin) * NCH + ch_a).astype(np.int64)
    cnt = np.bincount(flat, minlength=NCORES * NW * NCH).reshape(NCORES, NW, NCH)
    ngrp = np.ceil(cnt.max(axis=0) / P).astype(np.int64)  # [NW, NCH]
    ngrp_flat = np.concatenate([ngrp[w] for w in range(NW)])  # (w, ch) order
    NG = int(ngrp.sum())

    # sort: bucket-major, then src within bucket (HBM locality for gathers)
    order_all = np.lexsort((src_a, flat))
    bounds = np.searchsorted(flat[order_all], np.arange(NCORES * NW * NCH + 1))

    # padded stream layout per core, (w, ch) bucket order; bucket (w, ch)
    # occupies ngrp[w, ch]*P slots. Stream offsets per bucket:
    pad_len = ngrp * P                         # [NW, NCH]
    # per-chunk stream lengths (for gather calls): L[ch] = sum_w ngrp[w,ch]*P
    L = [int(ngrp[:, ch].sum()) * P for ch in range(NCH)]

    cores = []
    for c in range(NCORES):
        # per-chunk local row ids in (w, g) order; flat group order (w, ch, g)
        idx_parts = [[] for _ in range(NCH)]
        # flat padded stream (group-major) for one-hot construction
        slot_stream = np.zeros(NG * P, dtype=np.int64)
        wgt_stream = np.zeros(NG * P, dtype=np.float32)
        goff = 0
        for w in range(NW):
            for ch in range(NCH):
                b = (c * NW + w) * NCH + ch
                ee = order_all[bounds[b]:bounds[b + 1]]
                k = ee.shape[0]
                npadded = int(ngrp[w, ch]) * P
                loc = np.zeros(npadded, dtype=np.int64)
                loc[:k] = src_a[ee] - ch * CHROWS
                idx_parts[ch].append(loc)
                slot_stream[goff:goff + k] = slot_a[ee]
                wgt_stream[goff:goff + k] = w_a[ee]
                goff += npadded
        widx = np.concatenate(
            [_wrap_idx(np.concatenate(p)) if p else np.zeros((128, 0), np.int16)
             for p in idx_parts], axis=1)
        # weighted one-hot slab [P, NG*P]: position i of group g ->
        # partition i%P is wrong -- stream is group-major so position
        # pos = g*P + i, partition = i, column = g*P + slot
        pos = np.arange(NG * P)
        part = pos % P
        grp = pos // P
        ohw = np.zeros((P, NG * P), dtype=FP8)
        vals = wgt_stream.astype(FP8)
        ohw[part, grp * P + slot_stream] = vals
        cores.append((widx, ohw))
    return ngrp, L, NG, dinv.astype(np.float32), cores


def _build(ngrp, L, NG, has_bias, has_bias2, has_blin):
    """Build the SPMD bass program (same for all cores)."""
    nc = bacc.Bacc("TRN2", num_devices=NCORES, num_swdge_queues=4,
                   dynamic_dma_scratch_size=32768)
    f32 = mybir.dt.float32
    bf16 = mybir.dt.bfloat16
    fp8 = mybir.dt.float8e4

    Loff = np.concatenate([[0], np.cumsum(L)])
    Ltot16 = int(Loff[-1]) // 16

    # ---- I/O ----
    xT = nc.dram_tensor("xT", [P, NPAD], bf16, kind="ExternalInput")
    widx_d = nc.dram_tensor("widx", [P, Ltot16], mybir.dt.int16,
                            kind="ExternalInput")
    ohw_d = nc.dram_tensor("ohw", [P, NG * P], fp8, kind="ExternalInput")
    y0full_d = nc.dram_tensor("y0full", [N, H], bf16, kind="ExternalInput")
    ohb_d = nc.dram_tensor("ohb", [P, NW * P], bf16, kind="ExternalInput")
    dinv2bc_d = nc.dram_tensor("dinv2bc", [P, NPAD], bf16, kind="ExternalInput")
    cntinvb_d = nc.dram_tensor("cntinvb", [P, G], f32, kind="ExternalInput")
    Ws_d = [nc.dram_tensor(f"W{i}", [P, H], bf16, kind="ExternalInput")
            for i in range(3)]
    Wlin_d = nc.dram_tensor("Wlin", [P, C], f32, kind="ExternalInput")
    biasT_d = b2bc_d = blinb_d = None
    if has_bias:
        biasT_d = nc.dram_tensor("biasT", [P, 3], f32, kind="ExternalInput")
    if has_bias2:
        b2bc_d = nc.dram_tensor("b2bc", [P, H], f32, kind="ExternalInput")
    if has_blin:
        blinb_d = nc.dram_tensor("blinb", [P, C], f32, kind="ExternalInput")
    out_d = nc.dram_tensor("out", [G, C], f32, kind="ExternalOutput")

    y_local = nc.dram_tensor("y_local", [NP, H], bf16, kind="Internal")
    y_full = nc.dram_tensor("y_full", [N, H], bf16, kind="Internal",
                            addr_space="Shared")
    ar_in = nc.dram_tensor("ar_in", [P, G], f32, kind="Internal")
    ar_out = nc.dram_tensor("ar_out", [P, G], f32, kind="Internal",
                            addr_space="Shared")

    relu = mybir.ActivationFunctionType.Relu
    copyf = mybir.ActivationFunctionType.Copy

    with TileContext(nc) as tc:
        with ExitStack() as ctx:
            pers = ctx.enter_context(tc.tile_pool(name="pers", bufs=1))
            sy = ctx.enter_context(tc.tile_pool(name="sy", bufs=3))
            soh = ctx.enter_context(tc.tile_pool(name="soh", bufs=3))
            sep = ctx.enter_context(tc.tile_pool(name="sep", bufs=3))
            gpools = [ctx.enter_context(tc.tile_pool(name=f"gat{ch}", bufs=3))
                      for ch in range(NCH)]
            psy = ctx.enter_context(tc.tile_pool(name="psy", bufs=2, space="PSUM"))
            psa = ctx.enter_context(tc.tile_pool(name="psa", bufs=3, space="PSUM"))
            psp = ctx.enter_context(tc.tile_pool(name="psp", bufs=1, space="PSUM"))

            # ---- persistent tiles ----
            hT = pers.tile([P, NPAD], bf16)
            nc.sync.dma_start(out=hT[:], in_=xT[:])
            widx = pers.tile([P, Ltot16], mybir.dt.int16)
            nc.sync.dma_start(out=widx[:], in_=widx_d[:])
            ohb = pers.tile([P, NW * P], bf16)
            nc.sync.dma_start(out=ohb[:], in_=ohb_d[:])
            dinv2bc = pers.tile([P, NPAD], bf16)
            nc.sync.dma_start(out=dinv2bc[:], in_=dinv2bc_d[:])
            cntinvb = pers.tile([P, G], f32)
            nc.sync.dma_start(out=cntinvb[:], in_=cntinvb_d[:])
            Ws = []
            for i in range(3):
                t = pers.tile([P, H], bf16, tag=f"W{i}")
                nc.sync.dma_start(out=t[:], in_=Ws_d[i][:])
                Ws.append(t)
            Wlin = pers.tile([P, C], f32)
            nc.sync.dma_start(out=Wlin[:], in_=Wlin_d[:])
            biasT = b2bc = blinb = None
            if has_bias:
                biasT = pers.tile([P, 3], f32)
                nc.sync.dma_start(out=biasT[:], in_=biasT_d[:])
            if has_bias2:
                b2bc = pers.tile([P, H], f32)
                nc.sync.dma_start(out=b2bc[:], in_=b2bc_d[:])
            if has_blin:
                blinb = pers.tile([P, C], f32)
                nc.sync.dma_start(out=blinb[:], in_=blinb_d[:])

            # gather call schedule per chunk: list of (start, n) within chunk
            calls = []
            for ch in range(NCH):
                cs = []
                p = 0
                while p < L[ch]:
                    n = min(NI, L[ch] - p)
                    cs.append((p, n))
                    p += n
                calls.append(cs)

            for layer in range(NLAYERS):
                # ---- y = h @ W  -> y_local -> allgather ----
                # layer 0's y = x @ W0 is precomputed host-side (y0full)
                if layer > 0:
                    for w in range(NW):
                        py = psy.tile([P, H], f32, space="PSUM", tag="py")
                        nc.tensor.matmul(out=py[:],
                                         lhsT=hT[:, w * P:(w + 1) * P],
                                         rhs=Ws[layer][:], start=True,
                                         stop=True)
                        yt = sy.tile([P, H], bf16, tag="yt")
                        nc.scalar.activation(out=yt[:], in_=py[:], func=copyf)
                        rows = min(NP - w * P, P)
                        nc.sync.dma_start(out=y_local[w * P:w * P + rows, :],
                                          in_=yt[:rows, :])
                    nc.gpsimd.collective_compute(
                        "AllGather", mybir.AluOpType.bypass,
                        ins=[y_local[:]], outs=[y_full[:]],
                        replica_groups=[list(range(NCORES))],
                    )
                ysrc = y0full_d if layer == 0 else y_full

                # ---- edge gather + one-hot stream + segment-sum matmuls ----
                cur = [-1] * NCH          # current gather call per chunk
                gtile = [None] * NCH
                pos = [0] * NCH           # consumed rows within chunk stream

                def next_group(ch):
                    if cur[ch] < 0 or pos[ch] >= calls[ch][cur[ch]][0] + calls[ch][cur[ch]][1]:
                        cur[ch] += 1
                        start, n = calls[ch][cur[ch]]
                        c0 = (Loff[ch] + start) // 16
                        gt = gpools[ch].tile([P, NI // P, H], bf16, tag=f"g{ch}")
                        rows0 = ch * CHROWS
                        rows1 = min(rows0 + CHROWS, N)
                        nc.gpsimd.dma_gather(
                            out_ap=gt[:, :n // P, :],
                            in_ap=ysrc[rows0:rows1],
                            idxs_ap=widx[:, c0:c0 + n // 16],
                            num_idxs=n,
                            num_idxs_reg=n,
                            elem_size=H,
                            single_packet=False,
                            queue_num=ch,
                        )
                        gtile[ch] = gt
                    start, _ = calls[ch][cur[ch]]
                    t = (pos[ch] - start) // P
                    pos[ch] += P
                    return gtile[ch][:, t, :]

                ohw_cur = [-1]
                ohw_tile = [None]

                def next_ohw(gcol):
                    blk = gcol // OHK
                    if blk != ohw_cur[0]:
                        ohw_cur[0] = blk
                        c0 = blk * OHK * P
                        w_cols = min(OHK * P, NG * P - c0)
                        t = soh.tile([P, OHK * P], fp8, tag="oh")
                        nc.sync.dma_start(out=t[:, :w_cols],
                                          in_=ohw_d[:, c0:c0 + w_cols])
                        ohw_tile[0] = t
                    o = gcol % OHK
                    return ohw_tile[0][:, o * P:(o + 1) * P]

                gcol = 0
                pool_ps = None
                if layer == NLAYERS - 1:
                    pool_ps = psp.tile([P, G], f32, space="PSUM", tag="pp")
                for w in range(NW):
                    pa = psa.tile([P, P], f32, space="PSUM", tag="pa")
                    ng_w = int(ngrp[w].sum())
                    # self-loop term: (hT_w * dinv^2) @ W opens the PSUM chain
                    hts = sep.tile([P, P], bf16, tag="hts")
                    nc.vector.tensor_tensor(
                        out=hts[:], in0=hT[:, w * P:(w + 1) * P],
                        in1=dinv2bc[:, w * P:(w + 1) * P],
                        op=mybir.AluOpType.mult)
                    if layer < 2:
                        # reversed: out[f, slot] = W^T(fi,f) @ hts(fi, slot)
                        nc.tensor.matmul(
                            out=pa[:], lhsT=Ws[layer][:], rhs=hts[:],
                            start=True, stop=(ng_w == 0),
                            skip_group_check=True)
                    else:
                        # forward: out[slot, f] = hts^T(fi,slot) @ W(fi, f)
                        nc.tensor.matmul(
                            out=pa[:], lhsT=hts[:], rhs=Ws[layer][:],
                            start=True, stop=(ng_w == 0),
                            skip_group_check=True)
                    done = 0
                    for ch in range(NCH):
                        for g in range(int(ngrp[w, ch])):
                            ye = next_group(ch)
                            oh = next_ohw(gcol)
                            gcol += 1
                            done += 1
                            if layer < 2:
                                # reversed: out[f, slot]
                                nc.tensor.matmul(
                                    out=pa[:], lhsT=ye, rhs=oh,
                                    start=False, stop=(done == ng_w),
                                    skip_group_check=True)
                            else:
                                # forward: out[slot, f]
                                nc.tensor.matmul(
                                    out=pa[:], lhsT=oh, rhs=ye,
                                    start=False, stop=(done == ng_w),
                                    skip_group_check=True)

                    if layer < 2:
                        # epilogue on Scalar: relu(pa + b) -> hT window (bf16)
                        if has_bias:
                            nc.scalar.activation(
                                out=hT[:, w * P:(w + 1) * P], in_=pa[:],
                                func=relu, bias=biasT[:, layer:layer + 1])
                        else:
                            nc.scalar.activation(
                                out=hT[:, w * P:(w + 1) * P], in_=pa[:],
                                func=relu)
                    else:
                        h3 = sep.tile([P, H], bf16, tag="h3")
                        if has_bias2:
                            hb = sep.tile([P, H], f32, tag="hb")
                            nc.vector.tensor_tensor(
                                out=hb[:], in0=pa[:], in1=b2bc[:],
                                op=mybir.AluOpType.add)
                            nc.scalar.activation(out=h3[:], in_=hb[:], func=relu)
                        else:
                            nc.scalar.activation(out=h3[:], in_=pa[:], func=relu)
                        # pooling: poolT[f, g] += h3[n, f]^T @ ohb_w[n, g]
                        nc.tensor.matmul(
                            out=pool_ps[:], lhsT=h3[:],
                            rhs=ohb[:, w * P:(w + 1) * P],
                            start=(w == 0), stop=(w == NW - 1),
                            skip_group_check=True)

            # ---- pooling finish ----
            poolsb = sep.tile([P, G], f32, tag="poolsb")
            nc.vector.tensor_copy(out=poolsb[:], in_=pool_ps[:])
            nc.sync.dma_start(out=ar_in[:], in_=poolsb[:])
            nc.gpsimd.collective_compute(
                "AllReduce", mybir.AluOpType.add,
                ins=[ar_in[:]], outs=[ar_out[:]],
                replica_groups=[list(range(NCORES))],
            )
            art = sep.tile([P, G], f32, tag="art")
            nc.sync.dma_start(out=art[:], in_=ar_out[:])
            ptile = sep.tile([P, G], f32, tag="ptile")
            nc.vector.tensor_tensor(out=ptile[:], in0=art[:], in1=cntinvb[:],
                                    op=mybir.AluOpType.mult)
            po = psy.tile([P, C], f32, space="PSUM", tag="po")
            nc.tensor.matmul(out=po[:], lhsT=ptile[:], rhs=Wlin[:],
                             start=True, stop=True)
            ot = sep.tile([P, C], f32, tag="ot")
            if has_blin:
                nc.vector.tensor_tensor(out=ot[:], in0=po[:], in1=blinb[:],
                                        op=mybir.AluOpType.add)
            else:
                nc.vector.tensor_copy(out=ot[:], in_=po[:])
            nc.sync.dma_start(out=out_d[:], in_=ot[:G, :])

    nc.compile()
    return nc


def kernel(x, edge_index, batch, W0, b0, W1, b1, W2, b2, Wlin, blin):
    x = np.asarray(x, dtype=np.float32)
    batch_np = np.asarray(batch, dtype=np.int64)
    Wl = [np.asarray(w, dtype=np.float32) for w in (W0, W1, W2)]
    bl = [np.asarray(b, dtype=np.float32) for b in (b0, b1, b2)]
    Wlin = np.asarray(Wlin, dtype=np.float32)
    blin = np.asarray(blin, dtype=np.float32)

    ngrp, L, NG, dinv, cores = _preprocess(np.asarray(edge_index))
    y0full = (x @ Wl[0]).astype(BF16)  # layer-0 y precomputed host-side
    has_bias = any(np.abs(b).max() > 0 for b in bl[:2])
    has_bias2 = bool(np.abs(bl[2]).max() > 0)
    has_blin = bool(np.abs(blin).max() > 0)

    cnt = np.bincount(batch_np, minlength=G).astype(np.float32)
    cntinv = (1.0 / np.maximum(cnt, 1.0)).astype(np.float32)
    cntinvb = np.tile(cntinv[None, :], (P, 1)).astype(np.float32)  # [P, G]

    in_maps = []
    for c in range(NCORES):
        widx, ohw = cores[c]
        lo = c * NP
        xTa = np.zeros((P, NPAD), dtype=BF16)
        xTa[:, :NP] = x[lo:lo + NP].T.astype(BF16)
        # batch one-hot slab [P, NW*P]: ohb[i, w*P + g] = (batch[lo+w*P+i]==g)
        ohb = np.zeros((P, NW * P), dtype=BF16)
        pos = np.arange(NP)
        wv = pos // P
        iv = pos % P
        ohb[iv, wv * P + batch_np[lo:lo + NP]] = BF16(1.0)
        d2 = np.zeros(NPAD, dtype=np.float32)
        d2[:NP] = dinv[lo:lo + NP] ** 2
        dinv2bc = np.tile(d2[None, :], (P, 1)).astype(BF16)
        m = {
            "xT": xTa, "widx": widx, "ohw": ohw, "ohb": ohb,
            "y0full": y0full, "dinv2bc": dinv2bc, "cntinvb": cntinvb,
            "W0": Wl[0].astype(BF16), "W1": Wl[1].astype(BF16),
            "W2": Wl[2].astype(BF16), "Wlin": Wlin,
        }
        if has_bias:
            m["biasT"] = np.stack([bl[0], bl[1], np.zeros(H, np.float32)],
                                  axis=1).astype(np.float32)
        if has_bias2:
            m["b2bc"] = np.tile(bl[2][None, :], (P, 1)).astype(np.float32)
        if has_blin:
            m["blinb"] = np.tile(blin[None, :], (P, 1)).astype(np.float32)
        in_maps.append(m)

    nc = _build(ngrp, L, NG, has_bias, has_bias2, has_blin)
    res = run_bass_kernel_spmd(nc, in_maps, core_ids=list(range(NCORES)),
                               trace=TRACE)
    global LAST_RESULTS
    LAST_RESULTS = res
    return res.results[0]["out"]


# revision 31
# speedup vs baseline: 3.4613x; 1.2380x over previous
"""Distributed GCN (3x GCNConv + global mean pool + linear) on 8 TRN2 cores.

Sharding: nodes partitioned contiguously across 8 cores; edges partitioned by
dst owner; per-layer node features (bf16) all-gathered to a replicated table
in each core's DRAM; per-edge features fetched with dma_gather (SWDGE);
segment-sum over dst done with precomputed *weighted* one-hot tiles (edge
norm dinv_src*dinv_dst baked in, self-loops appended as edges) streamed from
DRAM via HWDGE.  Layers 1-2 use the reversed matmul form (out [feat, node])
so the aggregation output directly feeds the next layer's lhsT; layer 3 uses
the forward form so pooling can consume [node, feat] tiles via a single
PSUM-accumulated one-hot matmul chain.  Epilogues run on the Scalar engine
(PSUM -> SBUF relu + cast).  The Vector engine is idle in steady state, so
SWDGE descriptor generation never contends for the shared SBUF port pair.
"""
import math
import numpy as np
import ml_dtypes
from contextlib import ExitStack

import concourse.bacc as bacc
import concourse.mybir as mybir
from concourse.tile import TileContext
from concourse.bass_utils import run_bass_kernel_spmd

P = 128
NCORES = 8
N = 100000
E = 1600000
H = 128
C = 10
G = 128
NP = N // NCORES            # 12500 nodes per core
NW = math.ceil(NP / P)      # 98 dst windows per core
NPAD = NW * P               # 12544 padded nodes per core
NCH = 4                     # gather chunks (int16 idx limit 32767 per chunk)
NPA = 49 * P                # first-half nodes per core (AllGather split)
NPB = NP - NPA              # second-half nodes per core
AREG = NCORES * NPA         # y_full rows holding all first halves
NI = 2048                   # indices per dma_gather call
OHK = 16                    # one-hot groups fetched per DMA
NLAYERS = 3                 # debug knob

BF16 = ml_dtypes.bfloat16
FP8 = ml_dtypes.float8_e4m3

TRACE = False               # set by test.py for profiling runs
LAST_RESULTS = {}           # debug: per-core raw results


def _wrap_idx(idx):
    """int16 gather index layout: [128, len/16], i -> [i%16, i//16], tiled x8."""
    n = idx.shape[0]
    assert n % 16 == 0
    w = idx.reshape(n // 16, 16).T.astype(np.int16)   # [16, n/16]
    return np.tile(w, (8, 1))                          # [128, n/16]


def _preprocess(edge_index):
    """Partition/sort/pad edges (incl. self-loops) and build per-core
    gather-index and weighted-one-hot slabs."""
    src0 = np.asarray(edge_index[0], dtype=np.int64)
    dst0 = np.asarray(edge_index[1], dtype=np.int64)

    deg = np.bincount(dst0, minlength=N).astype(np.float64) + 1.0
    dinv = 1.0 / np.sqrt(deg)

    # self-loop term xw * dinv^2 is handled on-device (scaled-hT matmul),
    # not as gather edges
    src_a = src0
    dst_a = dst0
    w_a = (dinv[src_a] * dinv[dst_a]).astype(np.float32)

    # y_full row layout (AllGather split): [A halves of all cores | B halves]
    ids = np.arange(N, dtype=np.int64)
    coreof = ids // NP
    off = ids % NP
    newpos = np.where(off < NPA, coreof * NPA + off,
                      AREG + coreof * NPB + (off - NPA))
    gp = newpos[src_a]          # gather position of each edge's src

    owner = dst_a // NP
    wwin = (dst_a - owner * NP) // P
    slot_a = (dst_a - owner * NP - wwin * P).astype(np.int64)

    # chunk boundaries: grid-search splits (each chunk <= 32767 rows) to
    # minimize total padded groups NG = sum ceil(max_core bucket / 128)
    cands = []
    for a in (25000, 26000, 27000, 28000, 29000, 30000, 31000, 32000, 32767):
        rest = N - 3 * a
        if 0 < rest <= 32767:
            cands.append([0, a, 2 * a, 3 * a, N])
    for a, b in ((32000, 30000), (31000, 29000), (30000, 28000),
                 (32767, 31000), (29000, 26000)):
        rest = N - 2 * a - b
        if 0 < rest <= 32767 and b <= 32767:
            cands.append([0, a, 2 * a, 2 * a + b, N])
    best = None
    for cb in cands:
        ch_c = np.searchsorted(cb[1:-1], gp, side="right")
        flat_c = ((owner * NW + wwin) * NCH + ch_c).astype(np.int64)
        cnt = np.bincount(flat_c, minlength=NCORES * NW * NCH).reshape(
            NCORES, NW, NCH)
        ng = np.ceil(cnt.max(axis=0) / P).astype(np.int64)
        tot = int(ng.sum())
        if best is None or tot < best[0]:
            best = (tot, cb, ch_c, ng)
    NG, CB, ch_a, ngrp = best
    flat = ((owner * NW + wwin) * NCH + ch_a).astype(np.int64)

    # sort: bucket-major, then src position within bucket (HBM locality)
    order_all = np.lexsort((gp, flat))
    bounds = np.searchsorted(flat[order_all], np.arange(NCORES * NW * NCH + 1))

    # padded stream layout per core, (w, ch) bucket order; bucket (w, ch)
    # occupies ngrp[w, ch]*P slots. Stream offsets per bucket:
    pad_len = ngrp * P                         # [NW, NCH]
    # per-chunk stream lengths (for gather calls): L[ch] = sum_w ngrp[w,ch]*P
    L = [int(ngrp[:, ch].sum()) * P for ch in range(NCH)]

    cores = []
    for c in range(NCORES):
        # per-chunk local row ids in (w, g) order; flat group order (w, ch, g)
        idx_parts = [[] for _ in range(NCH)]
        # flat padded stream (group-major) for one-hot construction
        slot_stream = np.zeros(NG * P, dtype=np.int64)
        wgt_stream = np.zeros(NG * P, dtype=np.float32)
        goff = 0
        for w in range(NW):
            for ch in range(NCH):
                b = (c * NW + w) * NCH + ch
                ee = order_all[bounds[b]:bounds[b + 1]]
                k = ee.shape[0]
                npadded = int(ngrp[w, ch]) * P
                loc = np.zeros(npadded, dtype=np.int64)
                loc[:k] = gp[ee] - CB[ch]
                idx_parts[ch].append(loc)
                slot_stream[goff:goff + k] = slot_a[ee]
                wgt_stream[goff:goff + k] = w_a[ee]
                goff += npadded
        widx = np.concatenate(
            [_wrap_idx(np.concatenate(p)) if p else np.zeros((128, 0), np.int16)
             for p in idx_parts], axis=1)
        # weighted one-hot slab [P, NG*P]: position i of group g ->
        # partition i%P is wrong -- stream is group-major so position
        # pos = g*P + i, partition = i, column = g*P + slot
        pos = np.arange(NG * P)
        part = pos % P
        grp = pos // P
        ohw = np.zeros((P, NG * P), dtype=FP8)
        vals = wgt_stream.astype(FP8)
        ohw[part, grp * P + slot_stream] = vals
        cores.append((widx, ohw))
    return ngrp, L, NG, CB, newpos, dinv.astype(np.float32), cores


def _build(ngrp, L, NG, CB, has_bias, has_bias2, has_blin):
    """Build the SPMD bass program (same for all cores)."""
    nc = bacc.Bacc("TRN2", num_devices=NCORES, num_swdge_queues=4,
                   dynamic_dma_scratch_size=32768)
    f32 = mybir.dt.float32
    bf16 = mybir.dt.bfloat16
    fp8 = mybir.dt.float8e4

    Loff = np.concatenate([[0], np.cumsum(L)])
    Ltot16 = int(Loff[-1]) // 16

    # ---- I/O ----
    xT = nc.dram_tensor("xT", [P, NPAD], bf16, kind="ExternalInput")
    widx_d = nc.dram_tensor("widx", [P, Ltot16], mybir.dt.int16,
                            kind="ExternalInput")
    ohw_d = nc.dram_tensor("ohw", [P, NG * P], fp8, kind="ExternalInput")
    y0full_d = nc.dram_tensor("y0full", [N, H], bf16, kind="ExternalInput")
    ohb_d = nc.dram_tensor("ohb", [P, NW * P], bf16, kind="ExternalInput")
    dinv2bc_d = nc.dram_tensor("dinv2bc", [P, NPAD], bf16, kind="ExternalInput")
    cntinvb_d = nc.dram_tensor("cntinvb", [P, G], f32, kind="ExternalInput")
    Ws_d = [nc.dram_tensor(f"W{i}", [P, H], bf16, kind="ExternalInput")
            for i in range(3)]
    Wlin_d = nc.dram_tensor("Wlin", [P, C], f32, kind="ExternalInput")
    biasT_d = b2bc_d = blinb_d = None
    if has_bias:
        biasT_d = nc.dram_tensor("biasT", [P, 3], f32, kind="ExternalInput")
    if has_bias2:
        b2bc_d = nc.dram_tensor("b2bc", [P, H], f32, kind="ExternalInput")
    if has_blin:
        blinb_d = nc.dram_tensor("blinb", [P, C], f32, kind="ExternalInput")
    out_d = nc.dram_tensor("out", [G, C], f32, kind="ExternalOutput")

    y_localA = nc.dram_tensor("y_localA", [NPA, H], bf16, kind="Internal")
    y_localB = nc.dram_tensor("y_localB", [NPB, H], bf16, kind="Internal")
    y_full = nc.dram_tensor("y_full", [N, H], bf16, kind="Internal",
                            addr_space="Shared")
    ar_in = nc.dram_tensor("ar_in", [P, G], f32, kind="Internal")
    ar_out = nc.dram_tensor("ar_out", [P, G], f32, kind="Internal",
                            addr_space="Shared")

    relu = mybir.ActivationFunctionType.Relu
    copyf = mybir.ActivationFunctionType.Copy

    with TileContext(nc) as tc:
        with ExitStack() as ctx:
            pers = ctx.enter_context(tc.tile_pool(name="pers", bufs=1))
            sy = ctx.enter_context(tc.tile_pool(name="sy", bufs=3))
            soh = ctx.enter_context(tc.tile_pool(name="soh", bufs=3))
            sep = ctx.enter_context(tc.tile_pool(name="sep", bufs=3))
            gpools = [ctx.enter_context(tc.tile_pool(name=f"gat{ch}", bufs=3))
                      for ch in range(NCH)]
            psy = ctx.enter_context(tc.tile_pool(name="psy", bufs=2, space="PSUM"))
            psa = ctx.enter_context(tc.tile_pool(name="psa", bufs=3, space="PSUM"))
            psp = ctx.enter_context(tc.tile_pool(name="psp", bufs=1, space="PSUM"))

            # ---- persistent tiles ----
            hT = pers.tile([P, NPAD], bf16)
            nc.sync.dma_start(out=hT[:], in_=xT[:])
            widx = pers.tile([P, Ltot16], mybir.dt.int16)
            nc.sync.dma_start(out=widx[:], in_=widx_d[:])
            ohb = pers.tile([P, NW * P], bf16)
            nc.sync.dma_start(out=ohb[:], in_=ohb_d[:])
            dinv2bc = pers.tile([P, NPAD], bf16)
            nc.sync.dma_start(out=dinv2bc[:], in_=dinv2bc_d[:])
            cntinvb = pers.tile([P, G], f32)
            nc.sync.dma_start(out=cntinvb[:], in_=cntinvb_d[:])
            Ws = []
            for i in range(3):
                t = pers.tile([P, H], bf16, tag=f"W{i}")
                nc.sync.dma_start(out=t[:], in_=Ws_d[i][:])
                Ws.append(t)
            Wlin = pers.tile([P, C], f32)
            nc.sync.dma_start(out=Wlin[:], in_=Wlin_d[:])
            biasT = b2bc = blinb = None
            if has_bias:
                biasT = pers.tile([P, 3], f32)
                nc.sync.dma_start(out=biasT[:], in_=biasT_d[:])
            if has_bias2:
                b2bc = pers.tile([P, H], f32)
                nc.sync.dma_start(out=b2bc[:], in_=b2bc_d[:])
            if has_blin:
                blinb = pers.tile([P, C], f32)
                nc.sync.dma_start(out=blinb[:], in_=blinb_d[:])

            # gather call schedule per chunk: list of (start, n) within chunk
            calls = []
            for ch in range(NCH):
                cs = []
                p = 0
                while p < L[ch]:
                    n = min(NI, L[ch] - p)
                    cs.append((p, n))
                    p += n
                calls.append(cs)

            for layer in range(NLAYERS):
                # ---- y = h @ W  -> y_local -> allgather ----
                # layer 0's y = x @ W0 is precomputed host-side (y0full)
                if layer > 0:
                    for w in range(NW):
                        py = psy.tile([P, H], f32, space="PSUM", tag="py")
                        nc.tensor.matmul(out=py[:],
                                         lhsT=hT[:, w * P:(w + 1) * P],
                                         rhs=Ws[layer][:], start=True,
                                         stop=True)
                        yt = sy.tile([P, H], bf16, tag="yt")
                        nc.scalar.activation(out=yt[:], in_=py[:], func=copyf)
                        if w < 49:
                            nc.sync.dma_start(
                                out=y_localA[w * P:(w + 1) * P, :],
                                in_=yt[:])
                            if w == 48:
                                nc.gpsimd.collective_compute(
                                    "AllGather", mybir.AluOpType.bypass,
                                    ins=[y_localA[:]],
                                    outs=[y_full[0:AREG]],
                                    replica_groups=[list(range(NCORES))],
                                )
                        else:
                            r0 = (w - 49) * P
                            rows = min(NPB - r0, P)
                            nc.sync.dma_start(
                                out=y_localB[r0:r0 + rows, :],
                                in_=yt[:rows, :])
                    nc.gpsimd.collective_compute(
                        "AllGather", mybir.AluOpType.bypass,
                        ins=[y_localB[:]], outs=[y_full[AREG:N]],
                        replica_groups=[list(range(NCORES))],
                    )
                ysrc = y0full_d if layer == 0 else y_full

                # ---- edge gather + one-hot stream + segment-sum matmuls ----
                cur = [-1] * NCH          # current gather call per chunk
                gtile = [None] * NCH
                pos = [0] * NCH           # consumed rows within chunk stream

                def next_group(ch):
                    if cur[ch] < 0 or pos[ch] >= calls[ch][cur[ch]][0] + calls[ch][cur[ch]][1]:
                        cur[ch] += 1
                        start, n = calls[ch][cur[ch]]
                        c0 = (Loff[ch] + start) // 16
                        gt = gpools[ch].tile([P, NI // P, H], bf16, tag=f"g{ch}")
                        rows0 = CB[ch]
                        rows1 = CB[ch + 1]
                        nc.gpsimd.dma_gather(
                            out_ap=gt[:, :n // P, :],
                            in_ap=ysrc[rows0:rows1],
                            idxs_ap=widx[:, c0:c0 + n // 16],
                            num_idxs=n,
                            num_idxs_reg=n,
                            elem_size=H,
                            single_packet=False,
                            queue_num=ch,
                        )
                        gtile[ch] = gt
                    start, _ = calls[ch][cur[ch]]
                    t = (pos[ch] - start) // P
                    pos[ch] += P
                    return gtile[ch][:, t, :]

                ohw_cur = [-1]
                ohw_tile = [None]

                def next_ohw(gcol):
                    blk = gcol // OHK
                    if blk != ohw_cur[0]:
                        ohw_cur[0] = blk
                        c0 = blk * OHK * P
                        w_cols = min(OHK * P, NG * P - c0)
                        t = soh.tile([P, OHK * P], fp8, tag="oh")
                        nc.sync.dma_start(out=t[:, :w_cols],
                                          in_=ohw_d[:, c0:c0 + w_cols])
                        ohw_tile[0] = t
                    o = gcol % OHK
                    return ohw_tile[0][:, o * P:(o + 1) * P]

                gcol = 0
                pool_ps = None
                if layer == NLAYERS - 1:
                    pool_ps = psp.tile([P, G], f32, space="PSUM", tag="pp")
                for w in range(NW):
                    pa = psa.tile([P, P], f32, space="PSUM", tag="pa")
                    ng_w = int(ngrp[w].sum())
                    # self-loop term: (hT_w * dinv^2) @ W opens the PSUM chain
                    hts = sep.tile([P, P], bf16, tag="hts")
                    nc.vector.tensor_tensor(
                        out=hts[:], in0=hT[:, w * P:(w + 1) * P],
                        in1=dinv2bc[:, w * P:(w + 1) * P],
                        op=mybir.AluOpType.mult)
                    if layer < 2:
                        # reversed: out[f, slot] = W^T(fi,f) @ hts(fi, slot)
                        nc.tensor.matmul(
                            out=pa[:], lhsT=Ws[layer][:], rhs=hts[:],
                            start=True, stop=(ng_w == 0),
                            skip_group_check=True)
                    else:
                        # forward: out[slot, f] = hts^T(fi,slot) @ W(fi, f)
                        nc.tensor.matmul(
                            out=pa[:], lhsT=hts[:], rhs=Ws[layer][:],
                            start=True, stop=(ng_w == 0),
                            skip_group_check=True)
                    done = 0
                    for ch in range(NCH):
                        for g in range(int(ngrp[w, ch])):
                            ye = next_group(ch)
                            oh = next_ohw(gcol)
                            gcol += 1
                            done += 1
                            if layer < 2:
                                # reversed: out[f, slot]
                                nc.tensor.matmul(
                                    out=pa[:], lhsT=ye, rhs=oh,
                                    start=False, stop=(done == ng_w),
                                    skip_group_check=True)
                            else:
                                # forward: out[slot, f]
                                nc.tensor.matmul(
                                    out=pa[:], lhsT=oh, rhs=ye,
                                    start=False, stop=(done == ng_w),
                                    skip_group_check=True)

                    if layer < 2:
                        # epilogue on Scalar: relu(pa + b) -> hT window (bf16)
                        if has_bias:
                            nc.scalar.activation(
                                out=hT[:, w * P:(w + 1) * P], in_=pa[:],
                                func=relu, bias=biasT[:, layer:layer + 1])
                        else:
                            nc.scalar.activation(
                                out=hT[:, w * P:(w + 1) * P], in_=pa[:],
                                func=relu)
                    else:
                        h3 = sep.tile([P, H], bf16, tag="h3")
                        if has_bias2:
                            hb = sep.tile([P, H], f32, tag="hb")
                            nc.vector.tensor_tensor(
                                out=hb[:], in0=pa[:], in1=b2bc[:],
                                op=mybir.AluOpType.add)
                            nc.scalar.activation(out=h3[:], in_=hb[:], func=relu)
                        else:
                            nc.scalar.activation(out=h3[:], in_=pa[:], func=relu)
                        # pooling: poolT[f, g] += h3[n, f]^T @ ohb_w[n, g]
                        nc.tensor.matmul(
                            out=pool_ps[:], lhsT=h3[:],
                            rhs=ohb[:, w * P:(w + 1) * P],
                            start=(w == 0), stop=(w == NW - 1),
                            skip_group_check=True)

            # ---- pooling finish ----
            poolsb = sep.tile([P, G], f32, tag="poolsb")
            nc.vector.tensor_copy(out=poolsb[:], in_=pool_ps[:])
            nc.sync.dma_start(out=ar_in[:], in_=poolsb[:])
            nc.gpsimd.collective_compute(
                "AllReduce", mybir.AluOpType.add,
                ins=[ar_in[:]], outs=[ar_out[:]],
                replica_groups=[list(range(NCORES))],
            )
            art = sep.tile([P, G], f32, tag="art")
            nc.sync.dma_start(out=art[:], in_=ar_out[:])
            ptile = sep.tile([P, G], f32, tag="ptile")
            nc.vector.tensor_tensor(out=ptile[:], in0=art[:], in1=cntinvb[:],
                                    op=mybir.AluOpType.mult)
            po = psy.tile([P, C], f32, space="PSUM", tag="po")
            nc.tensor.matmul(out=po[:], lhsT=ptile[:], rhs=Wlin[:],
                             start=True, stop=True)
            ot = sep.tile([P, C], f32, tag="ot")
            if has_blin:
                nc.vector.tensor_tensor(out=ot[:], in0=po[:], in1=blinb[:],
                                        op=mybir.AluOpType.add)
            else:
                nc.vector.tensor_copy(out=ot[:], in_=po[:])
            nc.sync.dma_start(out=out_d[:], in_=ot[:G, :])

    nc.compile()
    return nc


def kernel(x, edge_index, batch, W0, b0, W1, b1, W2, b2, Wlin, blin):
    x = np.asarray(x, dtype=np.float32)
    batch_np = np.asarray(batch, dtype=np.int64)
    Wl = [np.asarray(w, dtype=np.float32) for w in (W0, W1, W2)]
    bl = [np.asarray(b, dtype=np.float32) for b in (b0, b1, b2)]
    Wlin = np.asarray(Wlin, dtype=np.float32)
    blin = np.asarray(blin, dtype=np.float32)

    ngrp, L, NG, CB, newpos, dinv, cores = _preprocess(np.asarray(edge_index))
    y0 = (x @ Wl[0]).astype(BF16)      # layer-0 y precomputed host-side
    y0full = np.empty_like(y0)         # rows in y_full (A/B split) order
    y0full[newpos] = y0
    has_bias = any(np.abs(b).max() > 0 for b in bl[:2])
    has_bias2 = bool(np.abs(bl[2]).max() > 0)
    has_blin = bool(np.abs(blin).max() > 0)

    cnt = np.bincount(batch_np, minlength=G).astype(np.float32)
    cntinv = (1.0 / np.maximum(cnt, 1.0)).astype(np.float32)
    cntinvb = np.tile(cntinv[None, :], (P, 1)).astype(np.float32)  # [P, G]

    in_maps = []
    for c in range(NCORES):
        widx, ohw = cores[c]
        lo = c * NP
        xTa = np.zeros((P, NPAD), dtype=BF16)
        xTa[:, :NP] = x[lo:lo + NP].T.astype(BF16)
        # batch one-hot slab [P, NW*P]: ohb[i, w*P + g] = (batch[lo+w*P+i]==g)
        ohb = np.zeros((P, NW * P), dtype=BF16)
        pos = np.arange(NP)
        wv = pos // P
        iv = pos % P
        ohb[iv, wv * P + batch_np[lo:lo + NP]] = BF16(1.0)
        d2 = np.zeros(NPAD, dtype=np.float32)
        d2[:NP] = dinv[lo:lo + NP] ** 2
        dinv2bc = np.tile(d2[None, :], (P, 1)).astype(BF16)
        m = {
            "xT": xTa, "widx": widx, "ohw": ohw, "ohb": ohb,
            "y0full": y0full, "dinv2bc": dinv2bc, "cntinvb": cntinvb,
            "W0": Wl[0].astype(BF16), "W1": Wl[1].astype(BF16),
            "W2": Wl[2].astype(BF16), "Wlin": Wlin,
        }
        if has_bias:
            m["biasT"] = np.stack([bl[0], bl[1], np.zeros(H, np.float32)],
                                  axis=1).astype(np.float32)
        if has_bias2:
            m["b2bc"] = np.tile(bl[2][None, :], (P, 1)).astype(np.float32)
        if has_blin:
            m["blinb"] = np.tile(blin[None, :], (P, 1)).astype(np.float32)
        in_maps.append(m)

    nc = _build(ngrp, L, NG, CB, has_bias, has_bias2, has_blin)
    res = run_bass_kernel_spmd(nc, in_maps, core_ids=list(range(NCORES)),
                               trace=TRACE)
    global LAST_RESULTS
    LAST_RESULTS = res
    return res.results[0]["out"]
